# revision 1
# baseline (speedup 1.0000x reference)
"""Trainium2 Bass kernel for few-shot video retrieval (bidirectional chamfer
distance to class prototypes, global frame-level + segment-level, fused).

Contract: kernel(**inputs) takes the FULL unsharded inputs (numpy) and returns
the full outputs (tuple of 4 [4096, 64] float32 arrays), matching reference().

Sharding: data-parallel over the query axis across 8 NeuronCores; support
features / labels / fusion params replicated. Gather on host by concatenation.

Device-side algorithm per core (512 queries):
  - cast-DMA loads fp32->bf16 (SWDGE)
  - class prototypes via one-hot matmul (contracts the support dim, so the
    result comes out feature-major, i.e. already transposed for the main GEMM)
  - queries are transposed on the TensorEngine by multiplying with a permuted
    diagonal matrix diag(1/||q_t||), which folds L2 normalization in for free
  - sims = nq^T . nproto as [128 q, 64k*8s] PSUM tiles per query-frame t;
    chamfer = grouped free-dim max/sum reductions on DVE/ACT
  - segment path: window sums built with a constant block-diagonal matmul;
    segment norms via ones-vector matmuls (partition-dim reduction on PE)
"""

import sys

sys.path.insert(0, "/opt/trn_rl_repo")

import numpy as np
import ml_dtypes
from contextlib import ExitStack

import concourse.bass as bass
import concourse.bacc as bacc
import concourse.tile as tile
from concourse import mybir
from concourse.bass_utils import run_bass_kernel_spmd

# ---------------------------------------------------------------- problem dims
S, Q, T, D = 256, 4096, 8, 1024
K = 64                      # classes
NCORES = 8
QPC = Q // NCORES           # 512 queries per core
ROWS = QPC * T              # 4096 frame rows per core
NBLK = ROWS // 128          # 32 row-blocks of 128 (16 queries x 8 frames)
QB = 16                     # queries per row-block
NSL = QPC // 128            # 4 query-slices of 128 per core
BPS = NBLK // NSL           # 8 row-blocks per query-slice
DCH = D // 128              # 8 chunks of the feature dim
NW = 3                      # segment windows
WINDOWS = ((0, 4), (2, 6), (4, 8))
SCH = S // 128              # 2 support chunks
GQ = 4 * QB                 # 64 queries per seg-norm group (4 blocks)
NG = NBLK // 4              # 8 seg-norm groups

F32 = mybir.dt.float32
BF16 = mybir.dt.bfloat16
I32 = mybir.dt.int32
AF = mybir.ActivationFunctionType
ALU = mybir.AluOpType
AX = mybir.AxisListType


# ------------------------------------------------------------- host constants
def _host_constants():
    # Permuted identity: Iperm[p, c] = 1 iff c == (p % T) * QB + p // T, so
    # the PE transpose of a [128(q,t), d] block lands in (t, q) column order.
    iperm = np.zeros((128, 128), dtype=ml_dtypes.bfloat16)
    for p in range(128):
        iperm[p, (p % T) * QB + p // T] = 1
    # Window matrix: B[q*T+t, w*QB+q] = 1 if t in WINDOWS[w] (un-normalized
    # window sum; everything downstream is scale-invariant under l2norm).
    bmat = np.zeros((128, NW * QB), dtype=ml_dtypes.bfloat16)
    for q in range(QB):
        for t in range(T):
            for w, (s0, e0) in enumerate(WINDOWS):
                if s0 <= t < e0:
                    bmat[q * T + t, w * QB + q] = 1
    ones_col = np.ones((128, 1), dtype=ml_dtypes.bfloat16)
    ones_row_f = np.ones((1, 128), dtype=np.float32)
    return iperm, bmat, ones_col, ones_row_f


# ---------------------------------------------------------------- bass kernel
def build_nc():
    nc = bacc.Bacc("TRN2", target_bir_lowering=False, debug=False,
                   num_devices=NCORES)

    tf = nc.dram_tensor("tf", [NBLK, 128, D], F32, kind="ExternalInput")
    sf = nc.dram_tensor("sf", [SCH, 128, T * D], F32, kind="ExternalInput")
    lab = nc.dram_tensor("lab", [SCH, 128, 1], F32, kind="ExternalInput")
    fus = nc.dram_tensor("fus", [1, 3], F32, kind="ExternalInput")
    lsc = nc.dram_tensor("lsc", [1, 1], F32, kind="ExternalInput")
    iperm_d = nc.dram_tensor("iperm", [128, 128], BF16, kind="ExternalInput")
    bmat_d = nc.dram_tensor("bmat", [128, NW * QB], BF16, kind="ExternalInput")
    onec_d = nc.dram_tensor("onec", [128, 1], BF16, kind="ExternalInput")
    onerf_d = nc.dram_tensor("onerf", [1, 128], F32, kind="ExternalInput")

    o_fus = nc.dram_tensor("o_fus", [NSL, 128, K], F32, kind="ExternalOutput")
    o_glo = nc.dram_tensor("o_glo", [NSL, 128, K], F32, kind="ExternalOutput")
    o_s2q = nc.dram_tensor("o_s2q", [NSL, 128, K], F32, kind="ExternalOutput")
    o_q2s = nc.dram_tensor("o_q2s", [NSL, 128, K], F32, kind="ExternalOutput")

    with tile.TileContext(nc) as tc, ExitStack() as ctx:
        const = ctx.enter_context(tc.tile_pool(name="const", bufs=1))
        persist = ctx.enter_context(tc.tile_pool(name="persist", bufs=1))
        work = ctx.enter_context(tc.tile_pool(name="work", bufs=3))

        # ---------------- constants
        iperm = const.tile([128, 128], BF16)
        nc.sync.dma_start(iperm[:], iperm_d[:])
        bmat = const.tile([128, NW * QB], BF16)
        nc.sync.dma_start(bmat[:], bmat_d[:])
        onec = const.tile([128, 1], BF16)
        nc.sync.dma_start(onec[:], onec_d[:])
        onerf = const.tile([1, 128], F32)
        nc.sync.dma_start(onerf[:], onerf_d[:])

        # ---------------- fusion weights: fw = softmax(fus) * exp(lsc)
        with tc.tile_pool(name="psF", bufs=1, space="PSUM") as psF:
            fus_t = work.tile([1, 3], F32, tag="fus")
            nc.sync.dma_start(fus_t[:], fus[:])
            lsc_t = work.tile([1, 1], F32, tag="lsc")
            nc.sync.dma_start(lsc_t[:], lsc[:])
            fmax = work.tile([1, 1], F32, tag="fmax")
            nc.vector.tensor_reduce(fmax[:], fus_t[:], axis=AX.X, op=ALU.max)
            nfmax = work.tile([1, 1], F32, tag="nfmax")
            nc.vector.tensor_scalar(nfmax[:], fmax[:], -1.0, None, ALU.mult)
            fexp = work.tile([1, 3], F32, tag="fexp")
            fsum = work.tile([1, 1], F32, tag="fsum")
            nc.scalar.activation(fexp[:], fus_t[:], AF.Exp, bias=nfmax[:],
                                 accum_out=fsum[:])
            fdenr = work.tile([1, 1], F32, tag="fdenr")
            nc.vector.reciprocal(fdenr[:], fsum[:])
            elsc = work.tile([1, 1], F32, tag="elsc")
            nc.scalar.activation(elsc[:], lsc_t[:], AF.Exp)
            scl = work.tile([1, 1], F32, tag="scl")
            nc.vector.tensor_tensor(scl[:], fdenr[:], elsc[:], ALU.mult)
            fw = work.tile([1, 3], F32, tag="fw")
            nc.vector.tensor_scalar(fw[:], fexp[:], scl[:], None, ALU.mult)
            # spread to per-partition columns: [128, 3] = onerf.T @ fw
            fw_ps = psF.tile([128, 3], F32)
            nc.tensor.matmul(fw_ps[:], onerf[:], fw[:], start=True, stop=True)
            fwc = persist.tile([128, 3], F32)
            nc.vector.tensor_copy(fwc[:], fw_ps[:])

        # ---------------- prototypes (frame + segment), transposed layouts
        protoT_n = persist.tile([128, DCH * K * T], BF16)
        psegT_n = persist.tile([128, DCH * K * NW], BF16)
        with tc.tile_pool(name="proto_scratch", bufs=1) as pscr, \
             tc.tile_pool(name="psP", bufs=2, space="PSUM") as psP, \
             tc.tile_pool(name="psN", bufs=1, space="PSUM") as psN:
            # one-hot labels
            kiota = pscr.tile([128, K], I32)
            nc.gpsimd.iota(kiota[:], pattern=[[1, K]], base=0,
                           channel_multiplier=0)
            kiota_f = pscr.tile([128, K], F32)
            nc.vector.tensor_copy(kiota_f[:], kiota[:])
            oh = []
            for c in range(SCH):
                lab_c = pscr.tile([128, 1], F32, tag=f"lab{c}")
                nc.sync.dma_start(lab_c[:], lab[c])
                oh_c = pscr.tile([128, K], BF16, tag=f"oh{c}")
                nc.vector.tensor_scalar(oh_c[:], kiota_f[:], lab_c[:], None,
                                        ALU.is_equal)
                oh.append(oh_c)
            # support (cast to bf16)
            supp = []
            for c in range(SCH):
                s_c = pscr.tile([128, T * D], BF16, tag=f"supp{c}")
                nc.gpsimd.dma_start(s_c[:], sf[c])
                supp.append(s_c)

            # protoT_raw free layout: dch*(T*K) + t*K + k
            protoT_raw = pscr.tile([128, DCH * T * K], BF16)
            for dch in range(DCH):
                pp = psP.tile([128, T * K], F32, tag="protops")
                for t in range(T):
                    for c in range(SCH):
                        nc.tensor.matmul(
                            pp[:, t * K:(t + 1) * K],
                            supp[c][:, t * D + dch * 128:
                                    t * D + (dch + 1) * 128],
                            oh[c], start=(c == 0), stop=(c == SCH - 1))
                nc.scalar.copy(protoT_raw[:, dch * T * K:(dch + 1) * T * K],
                               pp[:])

            # frame-proto norms^2 over d (partitions x chunks) via ones-matmul
            psq = pscr.tile([128, DCH * T * K], BF16)
            nc.vector.tensor_tensor(psq[:], protoT_raw[:], protoT_raw[:],
                                    ALU.mult)
            pn_ps = psN.tile([1, T * K], F32, tag="pnorm")
            for dch in range(DCH):
                nc.tensor.matmul(pn_ps[:], onec[:],
                                 psq[:, dch * T * K:(dch + 1) * T * K],
                                 start=(dch == 0), stop=(dch == DCH - 1))
            pn = pscr.tile([1, T * K], F32, tag="pn")
            nc.vector.tensor_copy(pn[:], pn_ps[:])
            pninv = pscr.tile([1, T * K], F32, tag="pninv")
            nc.vector.reciprocal(pninv[:], pn[:])
            rp = pscr.tile([1, T * K], F32, tag="rp")
            nc.scalar.activation(rp[:], pninv[:], AF.Sqrt)

            # replicate rp down partitions via K=1 ones-matmul
            rp_ps = psP.tile([128, T * K], F32, tag="rpspread")
            nc.tensor.matmul(rp_ps[:], onerf[:], rp[:], start=True, stop=True)
            rp_rep = pscr.tile([128, T * K], F32, tag="rprep")
            nc.vector.tensor_copy(rp_rep[:], rp_ps[:])
            # normalized protos, reordered (t,k) -> (k*T+t)
            rp_b = rp_rep[:]
            for dch in range(DCH):
                src = protoT_raw[:, dch * T * K:(dch + 1) * T * K]
                src = src.rearrange("p (t k) -> p t k", t=T)
                dst = protoT_n[:, dch * K * T:(dch + 1) * K * T]
                dst = dst.rearrange("p (k t) -> p t k", t=T)
                rpb = rp_b.rearrange("p (t k) -> p t k", t=T)
                nc.vector.tensor_tensor(dst, src, rpb, ALU.mult)

            # segment prototypes: window sums over t of protoT_raw
            # pseg_raw free layout: dch*(NW*K) + w*K + k
            pseg_raw = pscr.tile([128, DCH * NW * K], BF16)
            pr3 = protoT_raw[:].rearrange("p (c t k) -> p c t k", c=DCH, t=T)
            ps3 = pseg_raw[:].rearrange("p (c w k) -> p c w k", c=DCH, w=NW)
            epair = pscr.tile([128, DCH * 4 * K], BF16)
            ep3 = epair[:].rearrange("p (c e k) -> p c e k", c=DCH, e=4)
            for e in range(4):
                nc.vector.tensor_tensor(ep3[:, :, e, :], pr3[:, :, 2 * e, :],
                                        pr3[:, :, 2 * e + 1, :], ALU.add)
            nc.vector.tensor_tensor(ps3[:, :, 0, :], ep3[:, :, 0, :],
                                    ep3[:, :, 1, :], ALU.add)
            nc.vector.tensor_tensor(ps3[:, :, 1, :], ep3[:, :, 1, :],
                                    ep3[:, :, 2, :], ALU.add)
            nc.vector.tensor_tensor(ps3[:, :, 2, :], ep3[:, :, 2, :],
                                    ep3[:, :, 3, :], ALU.add)

            psegsq = pscr.tile([128, DCH * NW * K], BF16)
            nc.vector.tensor_tensor(psegsq[:], pseg_raw[:], pseg_raw[:],
                                    ALU.mult)
            pswn_ps = psN.tile([1, NW * K], F32, tag="psnorm")
            for dch in range(DCH):
                nc.tensor.matmul(pswn_ps[:], onec[:],
                                 psegsq[:, dch * NW * K:(dch + 1) * NW * K],
                                 start=(dch == 0), stop=(dch == DCH - 1))
            pswn = pscr.tile([1, NW * K], F32, tag="pswn")
            nc.vector.tensor_copy(pswn[:], pswn_ps[:])
            pswninv = pscr.tile([1, NW * K], F32, tag="pswninv")
            nc.vector.reciprocal(pswninv[:], pswn[:])
            rps = pscr.tile([1, NW * K], F32, tag="rps")
            nc.scalar.activation(rps[:], pswninv[:], AF.Sqrt)

            rps_ps = psP.tile([128, NW * K], F32, tag="rpsspread")
            nc.tensor.matmul(rps_ps[:], onerf[:], rps[:], start=True,
                             stop=True)
            rps_rep = pscr.tile([128, NW * K], F32, tag="rpsrep")
            nc.vector.tensor_copy(rps_rep[:], rps_ps[:])
            rps_b = rps_rep[:]
            for dch in range(DCH):
                src = pseg_raw[:, dch * NW * K:(dch + 1) * NW * K]
                src = src.rearrange("p (w k) -> p w k", w=NW)
                dst = psegT_n[:, dch * K * NW:(dch + 1) * K * NW]
                dst = dst.rearrange("p (k w) -> p w k", w=NW)
                rpsb = rps_b.rearrange("p (w k) -> p w k", w=NW)
                nc.vector.tensor_tensor(dst, src, rpsb, ALU.mult)

        # ---------------- per-block: load q, norms, transpose+normalize, segs
        # qT_all free layout: dch*ROWS + t*QPC + b*QB + q
        qT_all = persist.tile([128, DCH * ROWS], BF16)
        qT_v = qT_all[:].rearrange("p (c t b q) -> p c t b q", c=DCH, t=T,
                                   b=NBLK)
        # seg_all free layout: dch*(NW*QPC) + w*QPC + b*QB + q
        seg_all = persist.tile([128, DCH * NW * QPC], BF16)
        seg_v = seg_all[:].rearrange("p (c w b q) -> p c w b q", c=DCH, w=NW,
                                     b=NBLK)
        # seg norms^2, one row, w-major: sn_all free = w*(NG*GQ) + g4*GQ + q64
        # (so the per-(slice, w) column block of 128 queries is contiguous)
        sn_all = persist.tile([1, NW * NG * GQ], F32)

        qpool = ctx.enter_context(tc.tile_pool(name="qpool", bufs=4))
        sqpool = ctx.enter_context(tc.tile_pool(name="sqpool", bufs=2))
        psT = ctx.enter_context(tc.tile_pool(name="psT", bufs=2, space="PSUM"))
        psS = ctx.enter_context(tc.tile_pool(name="psS", bufs=1, space="PSUM"))
        psG = ctx.enter_context(tc.tile_pool(name="psG", bufs=2, space="PSUM"))
        psN2 = ctx.enter_context(tc.tile_pool(name="psN2", bufs=1,
                                              space="PSUM"))

        def do_block(b, qn2, j):
            # one 128-row block: norms, transpose+normalize, segments
            g4 = b // 4
            qn = qn2[:, j * D:(j + 1) * D]
            ssq = work.tile([128, 1], F32, tag="ssq")
            sqscr = work.tile([128, D], BF16, tag="sqscr")
            nc.scalar.activation(sqscr[:], qn, AF.Square, accum_out=ssq[:])
            rqi = work.tile([128, 1], F32, tag="rqi")
            nc.vector.reciprocal(rqi[:], ssq[:])
            rq = work.tile([128, 1], F32, tag="rq")
            nc.scalar.activation(rq[:], rqi[:], AF.Sqrt)

            diag = work.tile([128, 128], BF16, tag="diag")
            nc.vector.tensor_scalar(diag[:], iperm[:], rq[:], None, ALU.mult)

            # PE transpose+normalize, 4 d-chunks per PSUM tile (2 tiles/block)
            for h in range(2):
                pq = psT.tile([128, 512], F32, tag="qtps")
                for jj in range(4):
                    dch = h * 4 + jj
                    nc.tensor.matmul(pq[:, jj * 128:(jj + 1) * 128],
                                     qn[:, dch * 128:(dch + 1) * 128],
                                     diag[:], start=True, stop=True)
                dst = qT_v[:, h * 4:(h + 1) * 4, :, b, :]
                src = pq[:].rearrange("p (c t q) -> p c t q", c=4, t=T)
                if b % 2 == 0:
                    nc.scalar.copy(dst, src)
                else:
                    nc.vector.tensor_copy(dst, src)

            # raw transposed segments
            pseg = psS.tile([128, DCH * NW * QB], F32, tag="segps")
            for dch in range(DCH):
                nc.tensor.matmul(
                    pseg[:, dch * NW * QB:(dch + 1) * NW * QB],
                    qn[:, dch * 128:(dch + 1) * 128],
                    bmat[:], start=True, stop=True)
            ssrc = pseg[:].rearrange("p (c w q) -> p c w q", c=DCH, w=NW)
            nc.vector.tensor_copy(seg_v[:, :, :, b, :], ssrc)
            # seg squares into per-group tile (for seg norms)
            if b % 4 == 0:
                sqg_tiles[g4] = sqpool.tile([128, DCH * NW * GQ], BF16,
                                            tag="sqg", name=f"sqg{g4}")
            sq_g = sqg_tiles[g4]
            sq_v = sq_g[:].rearrange("p (c w r q) -> p c w r q", c=DCH, w=NW,
                                     r=4)
            nc.scalar.activation(sq_v[:, :, :, b % 4, :], ssrc, AF.Square)

            if b % 4 == 3:
                sn_ps = psN2.tile([1, NW * GQ], F32, tag="snps")
                for dch in range(DCH):
                    nc.tensor.matmul(
                        sn_ps[:], onec[:],
                        sq_g[:, dch * NW * GQ:(dch + 1) * NW * GQ],
                        start=(dch == 0), stop=(dch == DCH - 1))
                sn_dst = sn_all[:].rearrange("p (w g q) -> p w g q", w=NW,
                                             g=NG)[:, :, g4, :]
                nc.vector.tensor_copy(
                    sn_dst, sn_ps[:].rearrange("p (w q) -> p w q", w=NW))

        # interleave: per query-slice, do its 8 blocks then its G epilogues,
        # so the TensorEngine runs slice g's G matmuls while DVE/ACT chew on
        # slice g+1's blocks.
        sqg_tiles = {}
        for g in range(NSL):
            for bp in range(BPS // 2):
                b0 = g * BPS + 2 * bp
                # one 1-MiB DMA for two blocks
                qn2 = qpool.tile([128, 2 * D], BF16, tag="qn")
                src = tf[:].rearrange("b p d -> p b d")[:, b0:b0 + 2, :]
                nc.gpsimd.dma_start(
                    qn2[:].rearrange("p (b d) -> p b d", b=2), src)
                do_block(b0, qn2, 0)
                do_block(b0 + 1, qn2, 1)
            # ---- global frame-level
            a_sum = work.tile([128, K], F32, tag="asum")
            mmax = work.tile([128, K * T], BF16, tag="mmax")
            for t in range(T):
                gp = psG.tile([128, K * T], F32, tag="gps")
                for dch in range(DCH):
                    nc.tensor.matmul(
                        gp[:],
                        qT_all[:, dch * ROWS + t * QPC + g * 128:
                               dch * ROWS + t * QPC + (g + 1) * 128],
                        protoT_n[:, dch * K * T:(dch + 1) * K * T],
                        start=(dch == 0), stop=(dch == DCH - 1))
                sim_t = work.tile([128, K * T], BF16, tag="simt")
                if t % 2 == 0:
                    nc.scalar.copy(sim_t[:], gp[:])
                else:
                    nc.vector.tensor_copy(sim_t[:], gp[:])
                a_t = work.tile([128, K], BF16, tag="at")
                nc.vector.tensor_reduce(
                    a_t[:], sim_t[:].rearrange("p (k s) -> p k s", s=T),
                    axis=AX.X, op=ALU.max)
                if t == 0:
                    nc.vector.tensor_copy(a_sum[:], a_t[:])
                    nc.vector.tensor_copy(mmax[:], sim_t[:])
                else:
                    nc.vector.tensor_tensor(a_sum[:], a_sum[:], a_t[:],
                                            ALU.add)
                    nc.vector.tensor_tensor(mmax[:], mmax[:], sim_t[:],
                                            ALU.max)
            msum = work.tile([128, K], F32, tag="msum")
            nc.vector.tensor_reduce(
                msum[:], mmax[:].rearrange("p (k s) -> p k s", s=T),
                axis=AX.X, op=ALU.add)
            # -global = a_sum + msum - 16
            oglo = work.tile([128, K], F32, tag="oglo")
            nc.vector.scalar_tensor_tensor(
                oglo[:], in0=a_sum[:], scalar=-16.0, in1=msum[:],
                op0=ALU.add, op1=ALU.add)
            nc.sync.dma_start(o_glo[g], oglo[:])

            # ---- segment-level: rqseg columns per w
            rqs_w = []
            for w in range(NW):
                lt = sn_all[:, w * NG * GQ + g * 128:
                            w * NG * GQ + (g + 1) * 128]  # [1, 128]
                rq_ps = psN2.tile([128, 1], F32, tag="rqps")
                nc.tensor.matmul(rq_ps[:], lt, onerf[:, 0:1], start=True,
                                 stop=True)
                snc = work.tile([128, 1], F32, tag="snc")
                nc.vector.tensor_copy(snc[:], rq_ps[:])
                sni = work.tile([128, 1], F32, tag="sni")
                nc.vector.reciprocal(sni[:], snc[:])
                rqs = work.tile([128, 1], F32, tag=f"rqs{w}")
                nc.scalar.activation(rqs[:], sni[:], AF.Sqrt)
                rqs_w.append(rqs)

            b_sum = work.tile([128, K], F32, tag="bsum")
            msg = work.tile([128, K * NW], BF16, tag="mseg")
            for w in range(NW):
                sp = psG.tile([128, K * NW], F32, tag="gps")
                for dch in range(DCH):
                    nc.tensor.matmul(
                        sp[:],
                        seg_all[:, dch * NW * QPC + w * QPC + g * 128:
                                dch * NW * QPC + w * QPC + (g + 1) * 128],
                        psegT_n[:, dch * K * NW:(dch + 1) * K * NW],
                        start=(dch == 0), stop=(dch == DCH - 1))
                sim_w = work.tile([128, K * NW], BF16, tag="simw")
                nc.scalar.activation(sim_w[:], sp[:], AF.Copy,
                                     scale=rqs_w[w][:])
                b_w = work.tile([128, K], BF16, tag="bw")
                nc.vector.tensor_reduce(
                    b_w[:], sim_w[:].rearrange("p (k s) -> p k s", s=NW),
                    axis=AX.X, op=ALU.max)
                if w == 0:
                    nc.vector.tensor_copy(b_sum[:], b_w[:])
                    nc.vector.tensor_copy(msg[:], sim_w[:])
                else:
                    nc.vector.tensor_tensor(b_sum[:], b_sum[:], b_w[:],
                                            ALU.add)
                    nc.vector.tensor_tensor(msg[:], msg[:], sim_w[:], ALU.max)
            msgs = work.tile([128, K], F32, tag="msgs")
            nc.vector.tensor_reduce(
                msgs[:], msg[:].rearrange("p (k s) -> p k s", s=NW),
                axis=AX.X, op=ALU.add)
            oq2s = work.tile([128, K], F32, tag="oq2s")
            nc.vector.tensor_scalar(oq2s[:], b_sum[:], -3.0, None, ALU.add)
            nc.sync.dma_start(o_q2s[g], oq2s[:])
            os2q = work.tile([128, K], F32, tag="os2q")
            nc.vector.tensor_scalar(os2q[:], msgs[:], -3.0, None, ALU.add)
            nc.sync.dma_start(o_s2q[g], os2q[:])

            # ---- fused (parts already negated): f0*oglo + f1*os2q + f2*oq2s
            tmp0 = work.tile([128, K], F32, tag="tmp0")
            nc.vector.tensor_scalar(tmp0[:], oglo[:], fwc[:, 0:1], None,
                                    ALU.mult)
            tmp1 = work.tile([128, K], F32, tag="tmp1")
            nc.vector.scalar_tensor_tensor(
                tmp1[:], in0=os2q[:], scalar=fwc[:, 1:2], in1=tmp0[:],
                op0=ALU.mult, op1=ALU.add)
            ofus = work.tile([128, K], F32, tag="ofus")
            nc.vector.scalar_tensor_tensor(
                ofus[:], in0=oq2s[:], scalar=fwc[:, 2:3], in1=tmp1[:],
                op0=ALU.mult, op1=ALU.add)
            nc.sync.dma_start(o_fus[g], ofus[:])

    nc.compile()
    return nc


_NC_CACHE = None


def _get_nc():
    global _NC_CACHE
    if _NC_CACHE is None:
        _NC_CACHE = build_nc()
    return _NC_CACHE


# ------------------------------------------------------------------ host side
def kernel(support_features, target_features, support_labels, logit_scale,
           fusion_logits):
    support_features = np.asarray(support_features, dtype=np.float32)
    target_features = np.asarray(target_features, dtype=np.float32)
    support_labels = np.asarray(support_labels, dtype=np.int32)
    logit_scale = np.asarray(logit_scale, dtype=np.float32)
    fusion_logits = np.asarray(fusion_logits, dtype=np.float32)

    iperm, bmat, ones_col, ones_row_f = _host_constants()
    sf_h = np.ascontiguousarray(support_features.reshape(SCH, 128, T * D))
    lab_h = np.ascontiguousarray(
        support_labels.astype(np.float32).reshape(SCH, 128, 1))
    fus_h = fusion_logits.reshape(1, 3)
    lsc_h = logit_scale.reshape(1, 1)

    in_maps = []
    for c in range(NCORES):
        tf_c = target_features[c * QPC:(c + 1) * QPC].reshape(NBLK, 128, D)
        in_maps.append({
            "tf": np.ascontiguousarray(tf_c),
            "sf": sf_h, "lab": lab_h, "fus": fus_h, "lsc": lsc_h,
            "iperm": iperm, "bmat": bmat, "onec": ones_col, "onerf": ones_row_f,
        })

    nc = _get_nc()
    res = run_bass_kernel_spmd(nc, in_maps, core_ids=list(range(NCORES)))

    outs = []
    for name in ("o_fus", "o_glo", "o_s2q", "o_q2s"):
        parts = [res.results[c][name].reshape(QPC, K) for c in range(NCORES)]
        outs.append(np.concatenate(parts, axis=0).astype(np.float32))
    return tuple(outs)


if __name__ == "__main__":
    rng = np.random.default_rng(0)
    ins = {
        "support_features": rng.standard_normal((S, T, D), dtype=np.float32),
        "target_features": rng.standard_normal((Q, T, D), dtype=np.float32),
        "support_labels": (np.arange(S) % K).astype(np.int32),
        "logit_scale": np.float32(0.0),
        "fusion_logits": np.zeros(3, np.float32),
    }
    outs = kernel(**ins)
    for o in outs:
        print(o.shape, o.dtype, float(o.mean()))



# revision 7
# speedup vs baseline: 2.5453x; 2.5453x over previous
"""Trainium2 Bass kernel for few-shot video retrieval (bidirectional chamfer
distance to class prototypes, global frame-level + segment-level, fused).

Contract: kernel(**inputs) takes the FULL unsharded inputs (numpy) and returns
the full outputs (tuple of 4 [4096, 64] float32 arrays), matching reference().

Sharding: data-parallel over the query axis across 8 NeuronCores; support
features / labels / fusion params replicated. Gather on host by concatenation.

Device-side algorithm per core (512 queries = 4 slices of 128):
  - all GEMM operands are fp8 e4m3; the main sims GEMM runs in DoubleRow mode
    (256-deep contraction per matmul), the segment GEMM in plain fp8 (FWL)
  - host pre-transposes queries to d-major layout, so no PE transposes at all
  - class prototypes via one-hot matmuls (contract the support dim on the PE,
    result is d-major = already in rhs layout); normalization multiplies use
    host-provided replicated 16/||proto|| rows
  - per-(q,t) 1/(16||q||) scales are folded into the ACT PSUM->SBUF copies
  - chamfer min/sum reductions = bf16 max/add halving trees on the DVE
    (tensor_tensor runs 2x on bf16; grouped tensor_reduce would be 1x)
"""

import sys

sys.path.insert(0, "/opt/trn_rl_repo")

import numpy as np
import ml_dtypes
from contextlib import ExitStack

import concourse.bass as bass
import concourse.bacc as bacc
import concourse.tile as tile
from concourse import mybir
from concourse.bass_utils import run_bass_kernel_spmd

# ---------------------------------------------------------------- problem dims
S, Q, T, D = 256, 4096, 8, 1024
K = 64                      # classes
NCORES = 8
QPC = Q // NCORES           # 512 queries per core
G = QPC // 128              # 4 query-slices of 128 per core
DCH = D // 128              # 8 chunks of the feature dim
DJ = DCH // 2               # 4 DoubleRow chunks (256-deep)
NW = 3                      # segment windows
WINDOWS = ((0, 4), (2, 6), (4, 8))
SCH = S // 128              # 2 support chunks
FSCALE = 16.0               # fp8 range scale folded into the norm factors

F32 = mybir.dt.float32
BF16 = mybir.dt.bfloat16
F8 = mybir.dt.float8e4
I32 = mybir.dt.int32
AF = mybir.ActivationFunctionType
ALU = mybir.AluOpType
AX = mybir.AxisListType
DR = mybir.MatmulPerfMode.DoubleRow

NP_F8 = ml_dtypes.float8_e4m3
NP_BF16 = ml_dtypes.bfloat16


# ---------------------------------------------------------------- bass kernel
def build_nc():
    nc = bacc.Bacc("TRN2", target_bir_lowering=False, debug=False,
                   num_devices=NCORES)

    # per-slice d-major queries: [p(d%128), j(d//256), o((d//128)%2), t, q]
    tf = nc.dram_tensor("tf", [G, 128, DJ * 2 * T * 128], F8,
                        kind="ExternalInput")
    # per-slice d-major window-summed segments: [p, dch, w, q]
    tseg = nc.dram_tensor("tseg", [G, 128, DCH * NW * 128], F8,
                          kind="ExternalInput")
    # s-major support: [c, s, (dch, t, d%128)]
    sf = nc.dram_tensor("sf", [SCH, 128, T * D], F8, kind="ExternalInput")
    lab = nc.dram_tensor("lab", [SCH, 128, 1], F32, kind="ExternalInput")
    rqv = nc.dram_tensor("rqv", [G, 128, T], F32, kind="ExternalInput")
    rqs = nc.dram_tensor("rqs", [G, 128, NW], F32, kind="ExternalInput")
    rprep = nc.dram_tensor("rprep", [128, T * K], BF16, kind="ExternalInput")
    rpsrep = nc.dram_tensor("rpsrep", [128, NW * K], BF16,
                            kind="ExternalInput")
    fus = nc.dram_tensor("fus", [1, 3], F32, kind="ExternalInput")
    lsc = nc.dram_tensor("lsc", [1, 1], F32, kind="ExternalInput")
    onerf = nc.dram_tensor("onerf", [1, 128], F32, kind="ExternalInput")

    o_fus = nc.dram_tensor("o_fus", [G, 128, K], F32, kind="ExternalOutput")
    o_glo = nc.dram_tensor("o_glo", [G, 128, K], F32, kind="ExternalOutput")
    o_s2q = nc.dram_tensor("o_s2q", [G, 128, K], F32, kind="ExternalOutput")
    o_q2s = nc.dram_tensor("o_q2s", [G, 128, K], F32, kind="ExternalOutput")

    with tile.TileContext(nc) as tc, ExitStack() as ctx:
        const = ctx.enter_context(tc.tile_pool(name="const", bufs=1))
        persist = ctx.enter_context(tc.tile_pool(name="persist", bufs=1))
        work = ctx.enter_context(tc.tile_pool(name="work", bufs=3))

        # ---------------- small constants (front of the DMA queue)
        rprep_t = const.tile([128, T * K], BF16)
        nc.sync.dma_start(rprep_t[:], rprep[:])
        rpsrep_t = const.tile([128, NW * K], BF16)
        nc.sync.dma_start(rpsrep_t[:], rpsrep[:])
        onerf_t = const.tile([1, 128], F32)
        nc.sync.dma_start(onerf_t[:], onerf[:])
        rqv_t = []
        for g in range(G):
            t_ = const.tile([128, T], F32, name=f"rqv{g}")
            nc.sync.dma_start(t_[:], rqv[g])
            rqv_t.append(t_)
        rqs_t = []
        for g in range(G):
            t_ = const.tile([128, NW], F32, name=f"rqs{g}")
            nc.sync.dma_start(t_[:], rqs[g])
            rqs_t.append(t_)
        lab_t = []
        for c in range(SCH):
            t_ = const.tile([128, 1], F32, name=f"lab{c}")
            nc.sync.dma_start(t_[:], lab[c])
            lab_t.append(t_)

        # ---------------- support (dch-chunked so proto matmuls start early)
        supp = []
        for c in range(SCH):
            s_c = const.tile([128, T * D], F8, name=f"supp{c}")
            supp.append(s_c)
        for half in range(4):
            cols = slice(half * 2048, (half + 1) * 2048)
            for c in range(SCH):
                nc.sync.dma_start(supp[c][:, cols], sf[c][:, cols])

        # ---------------- big query tensors (gpsimd queue, parallel ring)
        tf_t = []
        tseg_t = []
        for g in range(G):
            q_ = const.tile([128, DJ * 2 * T * 128], F8, name=f"tf{g}")
            nc.gpsimd.dma_start(q_[:], tf[g])
            tf_t.append(q_)
            sg_ = const.tile([128, DCH * NW * 128], F8, name=f"tseg{g}")
            nc.gpsimd.dma_start(sg_[:], tseg[g])
            tseg_t.append(sg_)

        # ---------------- fusion weights: fw = softmax(fus) * exp(lsc)
        fwc = persist.tile([128, 3], F32)
        with tc.tile_pool(name="psF", bufs=1, space="PSUM") as psF:
            fus_t = work.tile([1, 3], F32, tag="fus")
            nc.sync.dma_start(fus_t[:], fus[:])
            lsc_t = work.tile([1, 1], F32, tag="lsc")
            nc.sync.dma_start(lsc_t[:], lsc[:])
            fmax = work.tile([1, 1], F32, tag="fmax")
            nc.vector.tensor_reduce(fmax[:], fus_t[:], axis=AX.X, op=ALU.max)
            nfmax = work.tile([1, 1], F32, tag="nfmax")
            nc.vector.tensor_scalar(nfmax[:], fmax[:], -1.0, None, ALU.mult)
            fexp = work.tile([1, 3], F32, tag="fexp")
            fsum = work.tile([1, 1], F32, tag="fsum")
            nc.scalar.activation(fexp[:], fus_t[:], AF.Exp, bias=nfmax[:],
                                 accum_out=fsum[:])
            fdenr = work.tile([1, 1], F32, tag="fdenr")
            nc.vector.reciprocal(fdenr[:], fsum[:])
            elsc = work.tile([1, 1], F32, tag="elsc")
            nc.scalar.activation(elsc[:], lsc_t[:], AF.Exp)
            scl = work.tile([1, 1], F32, tag="scl")
            nc.vector.tensor_tensor(scl[:], fdenr[:], elsc[:], ALU.mult)
            fw = work.tile([1, 3], F32, tag="fw")
            nc.vector.tensor_scalar(fw[:], fexp[:], scl[:], None, ALU.mult)
            fw_ps = psF.tile([128, 3], F32)
            nc.tensor.matmul(fw_ps[:], onerf_t[:], fw[:], start=True,
                             stop=True)
            nc.vector.tensor_copy(fwc[:], fw_ps[:])

        # ---------------- prototypes (frame + segment), d-major fp8
        # protoT free layout: [j(4), o(2), ts(8), k(64)]; value = 16*nproto
        protoT = persist.tile([128, DJ * 2 * T * K], F8)
        protoT_v = protoT[:].rearrange("p (j o s k) -> p j o s k", j=DJ, o=2,
                                       s=T)
        # npsegT free layout: [dch(8), ws(3), k(64)]; value = 16*npseg
        npsegT = persist.tile([128, DCH * NW * K], F8)
        npsegT_v = npsegT[:].rearrange("p (c w k) -> p c w k", c=DCH, w=NW)
        praw = persist.tile([128, DCH * T * K], BF16)

        with tc.tile_pool(name="pscr", bufs=1) as pscr, \
             tc.tile_pool(name="psP", bufs=2, space="PSUM") as psP:
            # one-hot labels (fp8: exact 0/1)
            kiota = pscr.tile([128, K], I32)
            nc.gpsimd.iota(kiota[:], pattern=[[1, K]], base=0,
                           channel_multiplier=0)
            kiota_f = pscr.tile([128, K], F32)
            nc.vector.tensor_copy(kiota_f[:], kiota[:])
            oh = []
            for c in range(SCH):
                oh_c = pscr.tile([128, K], F8, tag=f"oh{c}")
                nc.vector.tensor_scalar(oh_c[:], kiota_f[:], lab_t[c][:],
                                        None, ALU.is_equal)
                oh.append(oh_c)

            for dch in range(DCH):
                pp = psP.tile([128, T * K], F32, tag="pp")
                for t in range(T):
                    for c in range(SCH):
                        nc.tensor.matmul(
                            pp[:, t * K:(t + 1) * K],
                            supp[c][:, dch * (T * 128) + t * 128:
                                    dch * (T * 128) + (t + 1) * 128],
                            oh[c], start=(c == 0), stop=(c == SCH - 1))
                # normalized+scaled fp8 protos (critical path); note the
                # (j, o) block of protoT is contiguous at dch*T*K
                nc.vector.tensor_tensor(
                    protoT[:, dch * T * K:(dch + 1) * T * K],
                    pp[:], rprep_t[:], ALU.mult)
                # raw bf16 copy for the segment prototypes (off critical path)
                nc.scalar.copy(praw[:, dch * T * K:(dch + 1) * T * K], pp[:])

            # segment prototypes: window sums over ts of praw
            praw_v = praw[:].rearrange("p (c s k) -> p c s k", c=DCH, s=T)
            ep = pscr.tile([128, DCH * 4 * K], BF16)
            ep_v = ep[:].rearrange("p (c e k) -> p c e k", c=DCH, e=4)
            for e in range(4):
                nc.vector.tensor_tensor(ep_v[:, :, e, :],
                                        praw_v[:, :, 2 * e, :],
                                        praw_v[:, :, 2 * e + 1, :], ALU.add)
            psg = pscr.tile([128, DCH * NW * K], BF16)
            psg_v = psg[:].rearrange("p (c w k) -> p c w k", c=DCH, w=NW)
            for w in range(NW):
                nc.vector.tensor_tensor(psg_v[:, :, w, :],
                                        ep_v[:, :, w, :],
                                        ep_v[:, :, w + 1, :], ALU.add)
            for dch in range(DCH):
                nc.vector.tensor_tensor(
                    npsegT[:, dch * NW * K:(dch + 1) * NW * K],
                    psg[:, dch * NW * K:(dch + 1) * NW * K],
                    rpsrep_t[:], ALU.mult)

        # ---------------- main loop over the 4 query slices
        simpool = ctx.enter_context(tc.tile_pool(name="simpool", bufs=2))
        winpool = ctx.enter_context(tc.tile_pool(name="winpool", bufs=2))
        psM = ctx.enter_context(tc.tile_pool(name="psM", bufs=3, space="PSUM"))
        psS = ctx.enter_context(tc.tile_pool(name="psS", bufs=2, space="PSUM"))

        for g in range(G):
            tfg = tf_t[g][:].rearrange("p (j o t q) -> p j o t q", j=DJ, o=2,
                                       t=T)
            tsg = tseg_t[g][:].rearrange("p (c w q) -> p c w q", c=DCH, w=NW)

            # sims: [q, tq(8), ts(8), k(64)] bf16, true cosine values
            simcp = simpool.tile([128, T * T * K], BF16, tag="simcp")
            simv = simcp[:].rearrange("p (t s k) -> p t s k", t=T, s=T)
            for tq in range(T):
                mp = psM.tile([128, T * K], F32, tag="mp")
                for j in range(DJ):
                    nc.tensor.matmul(
                        mp[:], tfg[:, j, :, tq, :], protoT_v[:, j, :, :, :],
                        start=(j == 0), stop=(j == DJ - 1), perf_mode=DR)
                nc.scalar.activation(
                    simcp[:, tq * T * K:(tq + 1) * T * K], mp[:],
                    AF.Copy, scale=rqv_t[g][:, tq:tq + 1])

            # segment sims: [q, wq(3), ws(3), k(64)] bf16, normalized
            wins = winpool.tile([128, NW * NW * K], BF16, tag="wins")
            winv = wins[:].rearrange("p (v w k) -> p v w k", v=NW, w=NW)
            for wq in range(NW):
                sp = psS.tile([128, T * K], F32, tag="sp")
                for dch in range(DCH):
                    nc.tensor.matmul(
                        sp[:, 0:NW * K], tsg[:, dch, wq, :],
                        npsegT_v[:, dch, :, :], start=(dch == 0),
                        stop=(dch == DCH - 1))
                nc.scalar.activation(
                    wins[:, wq * NW * K:(wq + 1) * NW * K],
                    sp[:, 0:NW * K], AF.Copy, scale=rqs_t[g][:, wq:wq + 1])

            # ---- frame-level chamfer: halving max/add trees (bf16, 2x DVE)
            # dir2: max over tq (contiguous halves), then sum over ts
            m1 = work.tile([128, 2048], BF16, tag="m1")
            nc.vector.tensor_tensor(m1[:], simcp[:, 0:2048],
                                    simcp[:, 2048:4096], ALU.max)
            m2 = work.tile([128, 1024], BF16, tag="m2")
            nc.vector.tensor_tensor(m2[:], m1[:, 0:1024], m1[:, 1024:2048],
                                    ALU.max)
            mmax = work.tile([128, 512], BF16, tag="mmax")
            nc.vector.tensor_tensor(mmax[:], m2[:, 0:512], m2[:, 512:1024],
                                    ALU.max)
            s1 = work.tile([128, 256], BF16, tag="s1")
            nc.vector.tensor_tensor(s1[:], mmax[:, 0:256], mmax[:, 256:512],
                                    ALU.add)
            s2 = work.tile([128, 128], BF16, tag="s2")
            nc.vector.tensor_tensor(s2[:], s1[:, 0:128], s1[:, 128:256],
                                    ALU.add)
            msum = work.tile([128, K], BF16, tag="msum")
            nc.vector.tensor_tensor(msum[:], s2[:, 0:K], s2[:, K:128],
                                    ALU.add)
            # dir1: max over ts within each tq (strided), then sum over tq
            a1 = work.tile([128, 2048], BF16, tag="a1")
            a1v = a1[:].rearrange("p (t s k) -> p t s k", t=T, s=4)
            nc.vector.tensor_tensor(a1v, simv[:, :, 0:4, :],
                                    simv[:, :, 4:8, :], ALU.max)
            a2 = work.tile([128, 1024], BF16, tag="a2")
            a2v = a2[:].rearrange("p (t s k) -> p t s k", t=T, s=2)
            nc.vector.tensor_tensor(a2v, a1v[:, :, 0:2, :], a1v[:, :, 2:4, :],
                                    ALU.max)
            amax = work.tile([128, 512], BF16, tag="amax")
            amaxv = amax[:].rearrange("p (t k) -> p t k", t=T)
            nc.vector.tensor_tensor(amaxv, a2v[:, :, 0, :], a2v[:, :, 1, :],
                                    ALU.max)
            b1 = work.tile([128, 256], BF16, tag="b1")
            nc.vector.tensor_tensor(b1[:], amax[:, 0:256], amax[:, 256:512],
                                    ALU.add)
            b2 = work.tile([128, 128], BF16, tag="b2")
            nc.vector.tensor_tensor(b2[:], b1[:, 0:128], b1[:, 128:256],
                                    ALU.add)
            asum = work.tile([128, K], BF16, tag="asum")
            nc.vector.tensor_tensor(asum[:], b2[:, 0:K], b2[:, K:128],
                                    ALU.add)
            # -global = asum + msum - 16
            oglo = work.tile([128, K], F32, tag="oglo")
            nc.vector.scalar_tensor_tensor(
                oglo[:], in0=asum[:], scalar=-16.0, in1=msum[:],
                op0=ALU.add, op1=ALU.add)
            nc.sync.dma_start(o_glo[g], oglo[:])

            # ---- segment-level chamfer (tiny trees on wins)
            # q2s: max over ws within wq, sum over wq
            sa = work.tile([128, NW * K], BF16, tag="sa")
            sav = sa[:].rearrange("p (v k) -> p v k", v=NW)
            nc.vector.tensor_tensor(sav, winv[:, :, 0, :], winv[:, :, 1, :],
                                    ALU.max)
            nc.vector.tensor_tensor(sav, sav, winv[:, :, 2, :], ALU.max)
            st = work.tile([128, K], BF16, tag="st")
            nc.vector.tensor_tensor(st[:], sa[:, 0:K], sa[:, K:2 * K],
                                    ALU.add)
            oq2s = work.tile([128, K], F32, tag="oq2s")
            nc.vector.scalar_tensor_tensor(
                oq2s[:], in0=sa[:, 2 * K:3 * K], scalar=-3.0, in1=st[:],
                op0=ALU.add, op1=ALU.add)
            nc.sync.dma_start(o_q2s[g], oq2s[:])
            # s2q: max over wq, sum over ws
            sm = work.tile([128, NW * K], BF16, tag="sm")
            nc.vector.tensor_tensor(sm[:], wins[:, 0:NW * K],
                                    wins[:, NW * K:2 * NW * K], ALU.max)
            nc.vector.tensor_tensor(sm[:], sm[:],
                                    wins[:, 2 * NW * K:3 * NW * K], ALU.max)
            st2 = work.tile([128, K], BF16, tag="st2")
            nc.vector.tensor_tensor(st2[:], sm[:, 0:K], sm[:, K:2 * K],
                                    ALU.add)
            os2q = work.tile([128, K], F32, tag="os2q")
            nc.vector.scalar_tensor_tensor(
                os2q[:], in0=sm[:, 2 * K:3 * K], scalar=-3.0, in1=st2[:],
                op0=ALU.add, op1=ALU.add)
            nc.sync.dma_start(o_s2q[g], os2q[:])

            # ---- fused: f0*oglo + f1*os2q + f2*oq2s
            tmp0 = work.tile([128, K], F32, tag="tmp0")
            nc.vector.tensor_scalar(tmp0[:], oglo[:], fwc[:, 0:1], None,
                                    ALU.mult)
            tmp1 = work.tile([128, K], F32, tag="tmp1")
            nc.vector.scalar_tensor_tensor(
                tmp1[:], in0=os2q[:], scalar=fwc[:, 1:2], in1=tmp0[:],
                op0=ALU.mult, op1=ALU.add)
            ofus = work.tile([128, K], F32, tag="ofus")
            nc.vector.scalar_tensor_tensor(
                ofus[:], in0=oq2s[:], scalar=fwc[:, 2:3], in1=tmp1[:],
                op0=ALU.mult, op1=ALU.add)
            nc.sync.dma_start(o_fus[g], ofus[:])

    nc.compile()
    return nc


_NC_CACHE = None


def _get_nc():
    global _NC_CACHE
    if _NC_CACHE is None:
        _NC_CACHE = build_nc()
    return _NC_CACHE


# ------------------------------------------------------------------ host side
def build_in_maps(support_features, target_features, support_labels,
                  logit_scale, fusion_logits):
    support_features = np.asarray(support_features, dtype=np.float32)
    target_features = np.asarray(target_features, dtype=np.float32)
    support_labels = np.asarray(support_labels, dtype=np.int32)
    logit_scale = np.asarray(logit_scale, dtype=np.float32)
    fusion_logits = np.asarray(fusion_logits, dtype=np.float32)

    # ---- support: fp8 cast, s-major [c, s, (dch, t, d128)]
    s8 = support_features.astype(NP_F8)                    # [256, 8, 1024]
    sf_h = np.ascontiguousarray(
        s8.reshape(SCH, 128, T, DCH, 128).transpose(0, 1, 3, 2, 4)
    ).reshape(SCH, 128, T * D)
    lab_h = np.ascontiguousarray(
        support_labels.astype(np.float32).reshape(SCH, 128, 1))

    # ---- replicated proto norm rows (from the same fp8 values the device
    # accumulates, so they match the on-device prototype sums)
    s8f = s8.astype(np.float32)
    proto_sum = np.zeros((K, T, D), np.float32)
    np.add.at(proto_sum, support_labels % K, s8f)
    rp = FSCALE / np.sqrt((proto_sum * proto_sum).sum(-1))   # [K, T]
    rprep_h = np.ascontiguousarray(
        np.broadcast_to(rp.T.reshape(1, T * K), (128, T * K))).astype(NP_BF16)
    segp = np.stack([proto_sum[:, s:e].sum(1) for s, e in WINDOWS], 1)
    rps = FSCALE / np.sqrt((segp * segp).sum(-1))            # [K, NW]
    rpsrep_h = np.ascontiguousarray(
        np.broadcast_to(rps.T.reshape(1, NW * K),
                        (128, NW * K))).astype(NP_BF16)

    fus_h = fusion_logits.reshape(1, 3)
    lsc_h = logit_scale.reshape(1, 1)
    onerf_h = np.ones((1, 128), np.float32)

    in_maps = []
    for c in range(NCORES):
        x = target_features[c * QPC:(c + 1) * QPC]           # [512, 8, 1024]
        x8 = x.astype(NP_F8)
        x8f = x8.astype(np.float32)
        # d-major per-slice layout [g, p, j, o, t, q]
        tf_h = np.ascontiguousarray(
            x8.reshape(G, 128, T, DJ, 2, 128).transpose(0, 5, 3, 4, 2, 1)
        ).reshape(G, 128, DJ * 2 * T * 128)
        rqv_h = np.ascontiguousarray(
            (1.0 / (FSCALE * np.sqrt((x8f * x8f).sum(-1)))
             ).reshape(G, 128, T))
        # window-summed segments, re-cast to fp8, d-major [g, p, dch, w, q]
        segf = np.stack([x8f[:, s:e].sum(1) for s, e in WINDOWS], 1)
        seg8 = segf.astype(NP_F8)
        seg8f = seg8.astype(np.float32)
        tseg_h = np.ascontiguousarray(
            seg8.reshape(G, 128, NW, DCH, 128).transpose(0, 4, 3, 2, 1)
        ).reshape(G, 128, DCH * NW * 128)
        rqs_h = np.ascontiguousarray(
            (1.0 / (FSCALE * np.sqrt((seg8f * seg8f).sum(-1)))
             ).reshape(G, 128, NW))
        in_maps.append({
            "tf": tf_h, "tseg": tseg_h, "sf": sf_h, "lab": lab_h,
            "rqv": rqv_h, "rqs": rqs_h, "rprep": rprep_h, "rpsrep": rpsrep_h,
            "fus": fus_h, "lsc": lsc_h, "onerf": onerf_h,
        })
    return in_maps


def kernel(support_features, target_features, support_labels, logit_scale,
           fusion_logits):
    in_maps = build_in_maps(support_features, target_features, support_labels,
                            logit_scale, fusion_logits)
    nc = _get_nc()
    res = run_bass_kernel_spmd(nc, in_maps, core_ids=list(range(NCORES)))

    outs = []
    for name in ("o_fus", "o_glo", "o_s2q", "o_q2s"):
        parts = [res.results[c][name].reshape(QPC, K) for c in range(NCORES)]
        outs.append(np.concatenate(parts, axis=0).astype(np.float32))
    return tuple(outs)


if __name__ == "__main__":
    rng = np.random.default_rng(0)
    ins = {
        "support_features": rng.standard_normal((S, T, D), dtype=np.float32),
        "target_features": rng.standard_normal((Q, T, D), dtype=np.float32),
        "support_labels": (np.arange(S) % K).astype(np.int32),
        "logit_scale": np.float32(0.0),
        "fusion_logits": np.zeros(3, np.float32),
    }
    outs = kernel(**ins)
    for o in outs:
        print(o.shape, o.dtype, float(o.mean()))


# revision 16
# speedup vs baseline: 2.6467x; 1.0398x over previous
"""Trainium2 Bass kernel for few-shot video retrieval (bidirectional chamfer
distance to class prototypes, global frame-level + segment-level, fused).

Contract: kernel(**inputs) takes the FULL unsharded inputs (numpy) and returns
the full outputs (tuple of 4 [4096, 64] float32 arrays), matching reference().

Sharding: data-parallel over the query axis across 8 NeuronCores; support
features / labels / fusion params replicated. Gather on host by concatenation.

Device-side algorithm per core (512 queries = 4 slices of 128):
  - all GEMM operands are fp8 e4m3; the main sims GEMM runs in DoubleRow mode
    (256-deep contraction per matmul), the segment GEMM in plain fp8 (FWL)
  - host pre-transposes queries to d-major layout, so no PE transposes at all
  - class prototypes via one-hot matmuls (contract the support dim on the PE,
    result is d-major = already in rhs layout); normalization multiplies use
    host-provided replicated 16/||proto|| rows
  - per-(q,t) 1/(16||q||) scales are folded into the ACT PSUM->SBUF copies
  - chamfer min/sum reductions = bf16 max/add halving trees on the DVE
    (tensor_tensor runs 2x on bf16; grouped tensor_reduce would be 1x)
"""

import sys

sys.path.insert(0, "/opt/trn_rl_repo")

import numpy as np
import ml_dtypes
from contextlib import ExitStack

import concourse.bass as bass
import concourse.bacc as bacc
import concourse.tile as tile
from concourse import mybir
from concourse.bass_utils import run_bass_kernel_spmd

# ---------------------------------------------------------------- problem dims
S, Q, T, D = 256, 4096, 8, 1024
K = 64                      # classes
NCORES = 8
QPC = Q // NCORES           # 512 queries per core
G = QPC // 128              # 4 query-slices of 128 per core
DCH = D // 128              # 8 chunks of the feature dim
DJ = DCH // 2               # 4 DoubleRow chunks (256-deep)
NW = 3                      # segment windows
WINDOWS = ((0, 4), (2, 6), (4, 8))
SCH = S // 128              # 2 support chunks
FSCALE = 16.0               # fp8 range scale folded into the norm factors

F32 = mybir.dt.float32
BF16 = mybir.dt.bfloat16
F8 = mybir.dt.float8e4
I32 = mybir.dt.int32
AF = mybir.ActivationFunctionType
ALU = mybir.AluOpType
AX = mybir.AxisListType
DR = mybir.MatmulPerfMode.DoubleRow

NP_F8 = ml_dtypes.float8_e4m3
NP_BF16 = ml_dtypes.bfloat16


# ---------------------------------------------------------------- bass kernel
def build_nc():
    nc = bacc.Bacc("TRN2", target_bir_lowering=False, debug=False,
                   num_devices=NCORES)

    # per-slice d-major queries: [p(d%128), j(d//256), o((d//128)%2), t, q]
    tf = nc.dram_tensor("tf", [G, 128, DJ * 2 * T * 128], F8,
                        kind="ExternalInput")
    # per-slice d-major window-summed segments: [p, dch, w, q]
    tseg = nc.dram_tensor("tseg", [G, 128, DCH * NW * 128], F8,
                          kind="ExternalInput")
    # s-major support: [c, s, (dch, t, d%128)]
    sf = nc.dram_tensor("sf", [SCH, 128, T * D], F8, kind="ExternalInput")
    lab = nc.dram_tensor("lab", [SCH, 128, 1], F32, kind="ExternalInput")
    rqv = nc.dram_tensor("rqv", [G, 128, T], F32, kind="ExternalInput")
    rqs = nc.dram_tensor("rqs", [G, 128, NW], F32, kind="ExternalInput")
    rprep = nc.dram_tensor("rprep", [128, T * K], BF16, kind="ExternalInput")
    rpsrep = nc.dram_tensor("rpsrep", [128, NW * K], BF16,
                            kind="ExternalInput")
    fus = nc.dram_tensor("fus", [1, 3], F32, kind="ExternalInput")
    lsc = nc.dram_tensor("lsc", [1, 1], F32, kind="ExternalInput")
    onerf = nc.dram_tensor("onerf", [1, 128], F32, kind="ExternalInput")

    # one contiguous output blob: [p, (which(4), g(4), k(64))]
    oall = nc.dram_tensor("oall", [128, 4 * G * K], F32,
                          kind="ExternalOutput")

    NWARM = 40

    with tile.TileContext(nc) as tc, ExitStack() as ctx:
        const = ctx.enter_context(tc.tile_pool(name="const", bufs=1))
        persist = ctx.enter_context(tc.tile_pool(name="persist", bufs=1))
        work = ctx.enter_context(tc.tile_pool(name="work", bufs=3))

        # ---------------- PE warmup burst: flips the HAM clock gate to 8/8
        # while the input DMAs are still in flight (results never read)
        wz = const.tile([128, 512], F8)
        nc.gpsimd.memset(wz[:], 0)
        with tc.tile_pool(name="psW", bufs=1, space="PSUM") as psW:
            wps = psW.tile([128, 512], F32)
            for _ in range(NWARM):
                nc.tensor.matmul(wps[:], wz[:, 0:128], wz[:], start=True,
                                 stop=True)

        # ---------------- small constants (front of the DMA queue)
        rprep_t = const.tile([128, T * K], BF16)
        nc.sync.dma_start(rprep_t[:], rprep[:])
        rpsrep_t = const.tile([128, NW * K], BF16)
        nc.sync.dma_start(rpsrep_t[:], rpsrep[:])
        onerf_t = const.tile([1, 128], F32)
        nc.sync.dma_start(onerf_t[:], onerf[:])
        rqv_t = []
        for g in range(G):
            t_ = const.tile([128, T], F32, name=f"rqv{g}")
            nc.sync.dma_start(t_[:], rqv[g])
            rqv_t.append(t_)
        rqs_t = []
        for g in range(G):
            t_ = const.tile([128, NW], F32, name=f"rqs{g}")
            nc.sync.dma_start(t_[:], rqs[g])
            rqs_t.append(t_)
        lab_t = []
        for c in range(SCH):
            t_ = const.tile([128, 1], F32, name=f"lab{c}")
            nc.sync.dma_start(t_[:], lab[c])
            lab_t.append(t_)

        # ---------------- bulk data, all on the fast gpsimd DMA ring.
        # support first (in dch halves so the proto matmuls start early),
        # then the per-slice query tensors in consumption order.
        supp = []
        for c in range(SCH):
            s_c = const.tile([128, T * D], F8, name=f"supp{c}")
            supp.append(s_c)
        for half in range(2):
            cols = slice(half * 4096, (half + 1) * 4096)
            for c in range(SCH):
                nc.gpsimd.dma_start(supp[c][:, cols], sf[c][:, cols])

        tf_t = []
        tseg_t = []
        for g in range(G):
            q_ = const.tile([128, DJ * 2 * T * 128], F8, name=f"tf{g}")
            nc.gpsimd.dma_start(q_[:], tf[g])
            tf_t.append(q_)
            sg_ = const.tile([128, DCH * NW * 128], F8, name=f"tseg{g}")
            nc.gpsimd.dma_start(sg_[:], tseg[g])
            tseg_t.append(sg_)

        fus_t = const.tile([1, 3], F32, name="fus")
        nc.sync.dma_start(fus_t[:], fus[:])
        lsc_t = const.tile([1, 1], F32, name="lsc")
        nc.sync.dma_start(lsc_t[:], lsc[:])

        # ---------------- prototypes (frame + segment), d-major fp8
        # protoT free layout: [j(4), o(2), ts(8), k(64)]; value = 16*nproto
        protoT = persist.tile([128, DJ * 2 * T * K], F8)
        protoT_v = protoT[:].rearrange("p (j o s k) -> p j o s k", j=DJ, o=2,
                                       s=T)
        # npsegT free layout: [dch(8), ws(3), k(64)]; value = 16*npseg
        npsegT = persist.tile([128, DCH * NW * K], F8)
        npsegT_v = npsegT[:].rearrange("p (c w k) -> p c w k", c=DCH, w=NW)
        praw = persist.tile([128, DCH * T * K], BF16)

        with tc.tile_pool(name="pscr", bufs=1) as pscr, \
             tc.tile_pool(name="psP", bufs=2, space="PSUM") as psP:
            # one-hot labels (fp8: exact 0/1)
            kiota = pscr.tile([128, K], I32)
            nc.gpsimd.iota(kiota[:], pattern=[[1, K]], base=0,
                           channel_multiplier=0)
            kiota_f = pscr.tile([128, K], F32)
            nc.vector.tensor_copy(kiota_f[:], kiota[:])
            oh = []
            for c in range(SCH):
                oh_c = pscr.tile([128, K], F8, tag=f"oh{c}")
                nc.vector.tensor_scalar(oh_c[:], kiota_f[:], lab_t[c][:],
                                        None, ALU.is_equal)
                oh.append(oh_c)

            for dch in range(DCH):
                pp = psP.tile([128, T * K], F32, tag="pp")
                for t in range(T):
                    for c in range(SCH):
                        nc.tensor.matmul(
                            pp[:, t * K:(t + 1) * K],
                            supp[c][:, dch * (T * 128) + t * 128:
                                    dch * (T * 128) + (t + 1) * 128],
                            oh[c], start=(c == 0), stop=(c == SCH - 1))
                # normalized+scaled fp8 protos (critical path); note the
                # (j, o) block of protoT is contiguous at dch*T*K
                nc.vector.tensor_tensor(
                    protoT[:, dch * T * K:(dch + 1) * T * K],
                    pp[:], rprep_t[:], ALU.mult)
                # raw bf16 copy for the segment prototypes (off critical path)
                nc.scalar.copy(praw[:, dch * T * K:(dch + 1) * T * K], pp[:])

            # segment prototypes: window sums over ts of praw
            praw_v = praw[:].rearrange("p (c s k) -> p c s k", c=DCH, s=T)
            ep = pscr.tile([128, DCH * 4 * K], BF16)
            ep_v = ep[:].rearrange("p (c e k) -> p c e k", c=DCH, e=4)
            for e in range(4):
                nc.vector.tensor_tensor(ep_v[:, :, e, :],
                                        praw_v[:, :, 2 * e, :],
                                        praw_v[:, :, 2 * e + 1, :], ALU.add)
            psg = pscr.tile([128, DCH * NW * K], BF16)
            psg_v = psg[:].rearrange("p (c w k) -> p c w k", c=DCH, w=NW)
            for w in range(NW):
                nc.vector.tensor_tensor(psg_v[:, :, w, :],
                                        ep_v[:, :, w, :],
                                        ep_v[:, :, w + 1, :], ALU.add)
            for dch in range(DCH):
                nc.vector.tensor_tensor(
                    npsegT[:, dch * NW * K:(dch + 1) * NW * K],
                    psg[:, dch * NW * K:(dch + 1) * NW * K],
                    rpsrep_t[:], ALU.mult)

        # ---------------- fusion weights: fw = softmax(fus) * exp(lsc)
        # (after the proto matmuls in PE queue order so its late-arriving
        # inputs can never stall the PE FIFO)
        fwc = persist.tile([128, 3], F32)
        with tc.tile_pool(name="psF", bufs=1, space="PSUM") as psF:
            fmax = work.tile([1, 1], F32, tag="fmax")
            nc.vector.tensor_reduce(fmax[:], fus_t[:], axis=AX.X, op=ALU.max)
            nfmax = work.tile([1, 1], F32, tag="nfmax")
            nc.vector.tensor_scalar(nfmax[:], fmax[:], -1.0, None, ALU.mult)
            fexp = work.tile([1, 3], F32, tag="fexp")
            fsum = work.tile([1, 1], F32, tag="fsum")
            nc.scalar.activation(fexp[:], fus_t[:], AF.Exp, bias=nfmax[:],
                                 accum_out=fsum[:])
            fdenr = work.tile([1, 1], F32, tag="fdenr")
            nc.vector.reciprocal(fdenr[:], fsum[:])
            elsc = work.tile([1, 1], F32, tag="elsc")
            nc.scalar.activation(elsc[:], lsc_t[:], AF.Exp)
            scl = work.tile([1, 1], F32, tag="scl")
            nc.vector.tensor_tensor(scl[:], fdenr[:], elsc[:], ALU.mult)
            fw = work.tile([1, 3], F32, tag="fw")
            nc.vector.tensor_scalar(fw[:], fexp[:], scl[:], None, ALU.mult)
            fw_ps = psF.tile([128, 3], F32)
            nc.tensor.matmul(fw_ps[:], onerf_t[:], fw[:], start=True,
                             stop=True)
            nc.vector.tensor_copy(fwc[:], fw_ps[:])

        # ---------------- main loop over the 4 query slices
        obuf = persist.tile([128, 4 * G * K], F32)
        simpool = ctx.enter_context(tc.tile_pool(name="simpool", bufs=2))
        winpool = ctx.enter_context(tc.tile_pool(name="winpool", bufs=2))
        psM = ctx.enter_context(tc.tile_pool(name="psM", bufs=3, space="PSUM"))
        psS = ctx.enter_context(tc.tile_pool(name="psS", bufs=2, space="PSUM"))

        for g in range(G):
            tfg = tf_t[g][:].rearrange("p (j o t q) -> p j o t q", j=DJ, o=2,
                                       t=T)
            tsg = tseg_t[g][:].rearrange("p (c w q) -> p c w q", c=DCH, w=NW)

            # sims: [q, tq(8), ts(8), k(64)] bf16, true cosine values
            simcp = simpool.tile([128, T * T * K], BF16, tag="simcp")
            simv = simcp[:].rearrange("p (t s k) -> p t s k", t=T, s=T)
            for tq in range(T):
                mp = psM.tile([128, T * K], F32, tag="mp")
                for j in range(DJ):
                    nc.tensor.matmul(
                        mp[:], tfg[:, j, :, tq, :], protoT_v[:, j, :, :, :],
                        start=(j == 0), stop=(j == DJ - 1), perf_mode=DR)
                nc.scalar.activation(
                    simcp[:, tq * T * K:(tq + 1) * T * K], mp[:],
                    AF.Copy, scale=rqv_t[g][:, tq:tq + 1])

            # segment sims: [q, wq(3), ws(3), k(64)] bf16, normalized
            wins = winpool.tile([128, NW * NW * K], BF16, tag="wins")
            winv = wins[:].rearrange("p (v w k) -> p v w k", v=NW, w=NW)
            for wq in range(NW):
                sp = psS.tile([128, T * K], F32, tag="sp")
                for dch in range(DCH):
                    nc.tensor.matmul(
                        sp[:, 0:NW * K], tsg[:, dch, wq, :],
                        npsegT_v[:, dch, :, :], start=(dch == 0),
                        stop=(dch == DCH - 1))
                nc.scalar.activation(
                    wins[:, wq * NW * K:(wq + 1) * NW * K],
                    sp[:, 0:NW * K], AF.Copy, scale=rqs_t[g][:, wq:wq + 1])

            # ---- frame-level chamfer: halving max/add trees (bf16, 2x DVE)
            # dir2: max over tq (contiguous halves), then sum over ts
            m1 = work.tile([128, 2048], BF16, tag="m1")
            nc.vector.tensor_tensor(m1[:], simcp[:, 0:2048],
                                    simcp[:, 2048:4096], ALU.max)
            m2 = work.tile([128, 1024], BF16, tag="m2")
            nc.vector.tensor_tensor(m2[:], m1[:, 0:1024], m1[:, 1024:2048],
                                    ALU.max)
            mmax = work.tile([128, 512], BF16, tag="mmax")
            nc.vector.tensor_tensor(mmax[:], m2[:, 0:512], m2[:, 512:1024],
                                    ALU.max)
            s1 = work.tile([128, 256], BF16, tag="s1")
            nc.vector.tensor_tensor(s1[:], mmax[:, 0:256], mmax[:, 256:512],
                                    ALU.add)
            s2 = work.tile([128, 128], BF16, tag="s2")
            nc.vector.tensor_tensor(s2[:], s1[:, 0:128], s1[:, 128:256],
                                    ALU.add)
            msum = work.tile([128, K], BF16, tag="msum")
            nc.vector.tensor_tensor(msum[:], s2[:, 0:K], s2[:, K:128],
                                    ALU.add)
            # dir1: max over ts within each tq (strided), then sum over tq
            a1 = work.tile([128, 2048], BF16, tag="a1")
            a1v = a1[:].rearrange("p (t s k) -> p t s k", t=T, s=4)
            nc.vector.tensor_tensor(a1v, simv[:, :, 0:4, :],
                                    simv[:, :, 4:8, :], ALU.max)
            a2 = work.tile([128, 1024], BF16, tag="a2")
            a2v = a2[:].rearrange("p (t s k) -> p t s k", t=T, s=2)
            nc.vector.tensor_tensor(a2v, a1v[:, :, 0:2, :], a1v[:, :, 2:4, :],
                                    ALU.max)
            amax = work.tile([128, 512], BF16, tag="amax")
            amaxv = amax[:].rearrange("p (t k) -> p t k", t=T)
            nc.vector.tensor_tensor(amaxv, a2v[:, :, 0, :], a2v[:, :, 1, :],
                                    ALU.max)
            b1 = work.tile([128, 256], BF16, tag="b1")
            nc.vector.tensor_tensor(b1[:], amax[:, 0:256], amax[:, 256:512],
                                    ALU.add)
            b2 = work.tile([128, 128], BF16, tag="b2")
            nc.vector.tensor_tensor(b2[:], b1[:, 0:128], b1[:, 128:256],
                                    ALU.add)
            asum = work.tile([128, K], BF16, tag="asum")
            nc.vector.tensor_tensor(asum[:], b2[:, 0:K], b2[:, K:128],
                                    ALU.add)
            # -global = asum + msum - 16
            oglo = obuf[:, (G + g) * K:(G + g + 1) * K]
            nc.vector.scalar_tensor_tensor(
                oglo, in0=asum[:], scalar=-16.0, in1=msum[:],
                op0=ALU.add, op1=ALU.add)

            # ---- segment-level chamfer (tiny trees on wins)
            # q2s: max over ws within wq, sum over wq
            sa = work.tile([128, NW * K], BF16, tag="sa")
            sav = sa[:].rearrange("p (v k) -> p v k", v=NW)
            nc.vector.tensor_tensor(sav, winv[:, :, 0, :], winv[:, :, 1, :],
                                    ALU.max)
            nc.vector.tensor_tensor(sav, sav, winv[:, :, 2, :], ALU.max)
            st = work.tile([128, K], BF16, tag="st")
            nc.vector.tensor_tensor(st[:], sa[:, 0:K], sa[:, K:2 * K],
                                    ALU.add)
            oq2s = obuf[:, (3 * G + g) * K:(3 * G + g + 1) * K]
            nc.vector.scalar_tensor_tensor(
                oq2s, in0=sa[:, 2 * K:3 * K], scalar=-3.0, in1=st[:],
                op0=ALU.add, op1=ALU.add)
            # s2q: max over wq, sum over ws
            sm = work.tile([128, NW * K], BF16, tag="sm")
            nc.vector.tensor_tensor(sm[:], wins[:, 0:NW * K],
                                    wins[:, NW * K:2 * NW * K], ALU.max)
            nc.vector.tensor_tensor(sm[:], sm[:],
                                    wins[:, 2 * NW * K:3 * NW * K], ALU.max)
            st2 = work.tile([128, K], BF16, tag="st2")
            nc.vector.tensor_tensor(st2[:], sm[:, 0:K], sm[:, K:2 * K],
                                    ALU.add)
            os2q = obuf[:, (2 * G + g) * K:(2 * G + g + 1) * K]
            nc.vector.scalar_tensor_tensor(
                os2q, in0=sm[:, 2 * K:3 * K], scalar=-3.0, in1=st2[:],
                op0=ALU.add, op1=ALU.add)

            # ---- fused: f0*oglo + f1*os2q + f2*oq2s
            tmp0 = work.tile([128, K], F32, tag="tmp0")
            nc.vector.tensor_scalar(tmp0[:], oglo, fwc[:, 0:1], None,
                                    ALU.mult)
            tmp1 = work.tile([128, K], F32, tag="tmp1")
            nc.vector.scalar_tensor_tensor(
                tmp1[:], in0=os2q, scalar=fwc[:, 1:2], in1=tmp0[:],
                op0=ALU.mult, op1=ALU.add)
            ofus = obuf[:, g * K:(g + 1) * K]
            nc.vector.scalar_tensor_tensor(
                ofus, in0=oq2s, scalar=fwc[:, 2:3], in1=tmp1[:],
                op0=ALU.mult, op1=ALU.add)

        nc.gpsimd.dma_start(oall[:], obuf[:])

    nc.compile()
    return nc


_NC_CACHE = None


def _get_nc():
    global _NC_CACHE
    if _NC_CACHE is None:
        _NC_CACHE = build_nc()
    return _NC_CACHE


# ------------------------------------------------------------------ host side
def build_in_maps(support_features, target_features, support_labels,
                  logit_scale, fusion_logits):
    support_features = np.asarray(support_features, dtype=np.float32)
    target_features = np.asarray(target_features, dtype=np.float32)
    support_labels = np.asarray(support_labels, dtype=np.int32)
    logit_scale = np.asarray(logit_scale, dtype=np.float32)
    fusion_logits = np.asarray(fusion_logits, dtype=np.float32)

    # ---- support: fp8 cast, s-major [c, s, (dch, t, d128)]
    s8 = support_features.astype(NP_F8)                    # [256, 8, 1024]
    sf_h = np.ascontiguousarray(
        s8.reshape(SCH, 128, T, DCH, 128).transpose(0, 1, 3, 2, 4)
    ).reshape(SCH, 128, T * D)
    lab_h = np.ascontiguousarray(
        support_labels.astype(np.float32).reshape(SCH, 128, 1))

    # ---- replicated proto norm rows (from the same fp8 values the device
    # accumulates, so they match the on-device prototype sums)
    s8f = s8.astype(np.float32)
    proto_sum = np.zeros((K, T, D), np.float32)
    np.add.at(proto_sum, support_labels % K, s8f)
    rp = FSCALE / np.sqrt((proto_sum * proto_sum).sum(-1))   # [K, T]
    rprep_h = np.ascontiguousarray(
        np.broadcast_to(rp.T.reshape(1, T * K), (128, T * K))).astype(NP_BF16)
    segp = np.stack([proto_sum[:, s:e].sum(1) for s, e in WINDOWS], 1)
    rps = FSCALE / np.sqrt((segp * segp).sum(-1))            # [K, NW]
    rpsrep_h = np.ascontiguousarray(
        np.broadcast_to(rps.T.reshape(1, NW * K),
                        (128, NW * K))).astype(NP_BF16)

    fus_h = fusion_logits.reshape(1, 3)
    lsc_h = logit_scale.reshape(1, 1)
    onerf_h = np.ones((1, 128), np.float32)

    in_maps = []
    for c in range(NCORES):
        x = target_features[c * QPC:(c + 1) * QPC]           # [512, 8, 1024]
        x8 = x.astype(NP_F8)
        x8f = x8.astype(np.float32)
        # d-major per-slice layout [g, p, j, o, t, q]
        tf_h = np.ascontiguousarray(
            x8.reshape(G, 128, T, DJ, 2, 128).transpose(0, 5, 3, 4, 2, 1)
        ).reshape(G, 128, DJ * 2 * T * 128)
        rqv_h = np.ascontiguousarray(
            (1.0 / (FSCALE * np.sqrt((x8f * x8f).sum(-1)))
             ).reshape(G, 128, T))
        # window-summed segments, re-cast to fp8, d-major [g, p, dch, w, q]
        segf = np.stack([x8f[:, s:e].sum(1) for s, e in WINDOWS], 1)
        seg8 = segf.astype(NP_F8)
        seg8f = seg8.astype(np.float32)
        tseg_h = np.ascontiguousarray(
            seg8.reshape(G, 128, NW, DCH, 128).transpose(0, 4, 3, 2, 1)
        ).reshape(G, 128, DCH * NW * 128)
        rqs_h = np.ascontiguousarray(
            (1.0 / (FSCALE * np.sqrt((seg8f * seg8f).sum(-1)))
             ).reshape(G, 128, NW))
        in_maps.append({
            "tf": tf_h, "tseg": tseg_h, "sf": sf_h, "lab": lab_h,
            "rqv": rqv_h, "rqs": rqs_h, "rprep": rprep_h, "rpsrep": rpsrep_h,
            "fus": fus_h, "lsc": lsc_h, "onerf": onerf_h,
        })
    return in_maps


def kernel(support_features, target_features, support_labels, logit_scale,
           fusion_logits):
    in_maps = build_in_maps(support_features, target_features, support_labels,
                            logit_scale, fusion_logits)
    nc = _get_nc()
    res = run_bass_kernel_spmd(nc, in_maps, core_ids=list(range(NCORES)))

    outs = []
    for w in range(4):
        parts = []
        for c in range(NCORES):
            blob = np.asarray(res.results[c]["oall"]).reshape(128, 4, G, K)
            # [p, which, g, k] -> queries q = g*128 + p
            parts.append(blob[:, w].transpose(1, 0, 2).reshape(QPC, K))
        outs.append(np.concatenate(parts, axis=0).astype(np.float32))
    return tuple(outs)


if __name__ == "__main__":
    rng = np.random.default_rng(0)
    ins = {
        "support_features": rng.standard_normal((S, T, D), dtype=np.float32),
        "target_features": rng.standard_normal((Q, T, D), dtype=np.float32),
        "support_labels": (np.arange(S) % K).astype(np.int32),
        "logit_scale": np.float32(0.0),
        "fusion_logits": np.zeros(3, np.float32),
    }
    outs = kernel(**ins)
    for o in outs:
        print(o.shape, o.dtype, float(o.mean()))


# revision 23
# speedup vs baseline: 3.2554x; 1.2300x over previous
"""Trainium2 Bass kernel for few-shot video retrieval (bidirectional chamfer
distance to class prototypes, global frame-level + segment-level, fused).

Contract: kernel(**inputs) takes the FULL unsharded inputs (numpy) and returns
the full outputs (tuple of 4 [4096, 64] float32 arrays), matching reference().

Sharding: data-parallel over the query axis across 8 NeuronCores; support
features / labels / fusion params replicated. Gather on host by concatenation.

Device-side algorithm per core (512 queries = 4 slices of 128):
  - all GEMM operands are fp8 e4m3; the main sims GEMM runs in DoubleRow mode
    (256-deep contraction per matmul), the segment GEMM in plain fp8 (FWL)
  - host pre-transposes queries to d-major layout, so no PE transposes at all
  - class prototypes via one-hot matmuls (contract the support dim on the PE,
    result is d-major = already in rhs layout); normalization multiplies use
    host-provided replicated 16/||proto|| rows
  - per-(q,t) 1/(16||q||) scales are folded into the ACT PSUM->SBUF copies
  - chamfer min/sum reductions = bf16 max/add halving trees on the DVE
    (tensor_tensor runs 2x on bf16; grouped tensor_reduce would be 1x)
"""

import sys

sys.path.insert(0, "/opt/trn_rl_repo")

import numpy as np
import ml_dtypes
from contextlib import ExitStack

import concourse.bass as bass
import concourse.bacc as bacc
import concourse.tile as tile
from concourse import mybir
from concourse.bass_utils import run_bass_kernel_spmd

# ---------------------------------------------------------------- problem dims
S, Q, T, D = 256, 4096, 8, 1024
K = 64                      # classes
NCORES = 8
QPC = Q // NCORES           # 512 queries per core
G = QPC // 128              # 4 query-slices of 128 per core
DCH = D // 128              # 8 chunks of the feature dim
DJ = DCH // 2               # 4 DoubleRow chunks (256-deep)
NW = 3                      # segment windows
WINDOWS = ((0, 4), (2, 6), (4, 8))
SCH = S // 128              # 2 support chunks
FSCALE = 16.0               # fp8 range scale folded into the norm factors

F32 = mybir.dt.float32
BF16 = mybir.dt.bfloat16
F8 = mybir.dt.float8e4
I32 = mybir.dt.int32
AF = mybir.ActivationFunctionType
ALU = mybir.AluOpType
AX = mybir.AxisListType
DR = mybir.MatmulPerfMode.DoubleRow

NP_F8 = ml_dtypes.float8_e4m3
NP_BF16 = ml_dtypes.bfloat16


# ---------------------------------------------------------------- bass kernel
def build_nc():
    nc = bacc.Bacc("TRN2", target_bir_lowering=False, debug=False,
                   num_devices=NCORES)

    # per-slice d-major queries: [p(d%128), j(d//256), o((d//128)%2), t, q]
    tf = nc.dram_tensor("tf", [G, 128, DJ * 2 * T * 128], F8,
                        kind="ExternalInput")
    # per-slice d-major window-summed segments: [p, dch, w, q]
    tseg = nc.dram_tensor("tseg", [G, 128, DCH * NW * 128], F8,
                          kind="ExternalInput")
    # s-major support: [c, s, (dch, t, d%128)]
    sf = nc.dram_tensor("sf", [SCH, 128, T * D], F8, kind="ExternalInput")
    # packed per-partition f32 constants:
    # [lab0, lab1, rqv(g*8+t: 32), rqs(g*3+w: 12), kiota(64)]
    NSF = 2 + G * T + G * NW + K
    smallf = nc.dram_tensor("smallf", [128, NSF], F32, kind="ExternalInput")
    # packed bf16 rows: [rprep(512) | rpsrep(192)]
    smallb = nc.dram_tensor("smallb", [128, T * K + NW * K], BF16,
                            kind="ExternalInput")
    fus = nc.dram_tensor("fus", [1, 3], F32, kind="ExternalInput")
    lsc = nc.dram_tensor("lsc", [1, 1], F32, kind="ExternalInput")
    onerf = nc.dram_tensor("onerf", [1, 128], F32, kind="ExternalInput")

    # one contiguous output blob: [p, (which(4), g(4), k(64))]
    oall = nc.dram_tensor("oall", [128, 4 * G * K], F32,
                          kind="ExternalOutput")

    NWARM = 28

    with tile.TileContext(nc) as tc, ExitStack() as ctx:
        const = ctx.enter_context(tc.tile_pool(name="const", bufs=1))
        persist = ctx.enter_context(tc.tile_pool(name="persist", bufs=1))
        work = ctx.enter_context(tc.tile_pool(name="work", bufs=3))

        # ---------------- PE warmup burst: flips the HAM clock gate to 8/8
        # while the input DMAs are still in flight (results never read)
        wz = const.tile([128, 512], F8)
        nc.gpsimd.memset(wz[:], 0)
        with tc.tile_pool(name="psW", bufs=1, space="PSUM") as psW:
            wps = psW.tile([128, 512], F32)
            for _ in range(NWARM):
                nc.tensor.matmul(wps[:], wz[:, 0:128], wz[:], start=True,
                                 stop=True)

        # ---------------- packed small constants (head of the fast ring)
        smallf_t = const.tile([128, NSF], F32)
        nc.gpsimd.dma_start(smallf_t[:], smallf[:])
        smallb_t = const.tile([128, T * K + NW * K], BF16)
        nc.gpsimd.dma_start(smallb_t[:], smallb[:])
        lab_t = [smallf_t[:, c:c + 1] for c in range(SCH)]
        rqv_t = [smallf_t[:, 2 + g * T:2 + (g + 1) * T] for g in range(G)]
        rqs_t = [smallf_t[:, 2 + G * T + g * NW:2 + G * T + (g + 1) * NW]
                 for g in range(G)]
        kiota_f = smallf_t[:, 2 + G * T + G * NW:2 + G * T + G * NW + K]
        rprep_t = smallb_t[:, 0:T * K]
        rpsrep_t = smallb_t[:, T * K:T * K + NW * K]
        onerf_t = const.tile([1, 128], F32)
        nc.sync.dma_start(onerf_t[:], onerf[:])

        # ---------------- bulk data, all on the fast gpsimd DMA ring.
        # support first (in dch halves so the proto matmuls start early),
        # then the per-slice query tensors in consumption order.
        supp = []
        for c in range(SCH):
            s_c = const.tile([128, T * D], F8, name=f"supp{c}")
            supp.append(s_c)
        for half in range(2):
            cols = slice(half * 4096, (half + 1) * 4096)
            for c in range(SCH):
                nc.gpsimd.dma_start(supp[c][:, cols], sf[c][:, cols])

        tf_t = []
        tseg_t = []
        for g in range(G):
            q_ = const.tile([128, DJ * 2 * T * 128], F8, name=f"tf{g}")
            nc.gpsimd.dma_start(q_[:], tf[g])
            tf_t.append(q_)
            sg_ = const.tile([128, DCH * NW * 128], F8, name=f"tseg{g}")
            nc.gpsimd.dma_start(sg_[:], tseg[g])
            tseg_t.append(sg_)

        fus_t = const.tile([1, 3], F32, name="fus")
        nc.sync.dma_start(fus_t[:], fus[:])
        lsc_t = const.tile([1, 1], F32, name="lsc")
        nc.sync.dma_start(lsc_t[:], lsc[:])

        # ---------------- prototypes (frame + segment), d-major fp8
        # protoT free layout: [j(4), o(2), ts(8), k(64)]; value = 16*nproto
        protoT = persist.tile([128, DJ * 2 * T * K], F8)
        protoT_v = protoT[:].rearrange("p (j o s k) -> p j o s k", j=DJ, o=2,
                                       s=T)
        # npsegT free layout: [dch(8), ws(3), k(64)]; value = 16*npseg
        npsegT = persist.tile([128, DCH * NW * K], F8)
        npsegT_v = npsegT[:].rearrange("p (c w k) -> p c w k", c=DCH, w=NW)
        praw = persist.tile([128, DCH * T * K], BF16)

        with tc.tile_pool(name="pscr", bufs=1) as pscr, \
             tc.tile_pool(name="psP", bufs=2, space="PSUM") as psP:
            # one-hot labels (fp8: exact 0/1)
            oh = []
            for c in range(SCH):
                oh_c = pscr.tile([128, K], F8, tag=f"oh{c}")
                nc.vector.tensor_scalar(oh_c[:], kiota_f, lab_t[c],
                                        None, ALU.is_equal)
                oh.append(oh_c)

            for dch in range(DCH):
                pp = psP.tile([128, T * K], F32, tag="pp")
                for t in range(T):
                    for c in range(SCH):
                        nc.tensor.matmul(
                            pp[:, t * K:(t + 1) * K],
                            supp[c][:, dch * (T * 128) + t * 128:
                                    dch * (T * 128) + (t + 1) * 128],
                            oh[c], start=(c == 0), stop=(c == SCH - 1))
                # normalized+scaled fp8 protos (critical path); note the
                # (j, o) block of protoT is contiguous at dch*T*K
                nc.vector.tensor_tensor(
                    protoT[:, dch * T * K:(dch + 1) * T * K],
                    pp[:], rprep_t, ALU.mult)
                # raw bf16 copy for the segment prototypes (off critical path)
                nc.scalar.copy(praw[:, dch * T * K:(dch + 1) * T * K], pp[:])

            # segment prototypes: window sums over ts of praw
            praw_v = praw[:].rearrange("p (c s k) -> p c s k", c=DCH, s=T)
            ep = pscr.tile([128, DCH * 4 * K], BF16)
            ep_v = ep[:].rearrange("p (c e k) -> p c e k", c=DCH, e=4)
            for e in range(4):
                nc.vector.tensor_tensor(ep_v[:, :, e, :],
                                        praw_v[:, :, 2 * e, :],
                                        praw_v[:, :, 2 * e + 1, :], ALU.add)
            psg = pscr.tile([128, DCH * NW * K], BF16)
            psg_v = psg[:].rearrange("p (c w k) -> p c w k", c=DCH, w=NW)
            for w in range(NW):
                nc.vector.tensor_tensor(psg_v[:, :, w, :],
                                        ep_v[:, :, w, :],
                                        ep_v[:, :, w + 1, :], ALU.add)
            for dch in range(DCH):
                nc.vector.tensor_tensor(
                    npsegT[:, dch * NW * K:(dch + 1) * NW * K],
                    psg[:, dch * NW * K:(dch + 1) * NW * K],
                    rpsrep_t, ALU.mult)

        # ---------------- fusion weights: fw = softmax(fus) * exp(lsc)
        # (after the proto matmuls in PE queue order so its late-arriving
        # inputs can never stall the PE FIFO)
        fwc = persist.tile([128, 3], F32)
        with tc.tile_pool(name="psF", bufs=1, space="PSUM") as psF:
            fmax = work.tile([1, 1], F32, tag="fmax")
            nc.vector.tensor_reduce(fmax[:], fus_t[:], axis=AX.X, op=ALU.max)
            nfmax = work.tile([1, 1], F32, tag="nfmax")
            nc.vector.tensor_scalar(nfmax[:], fmax[:], -1.0, None, ALU.mult)
            fexp = work.tile([1, 3], F32, tag="fexp")
            fsum = work.tile([1, 1], F32, tag="fsum")
            nc.scalar.activation(fexp[:], fus_t[:], AF.Exp, bias=nfmax[:],
                                 accum_out=fsum[:])
            fdenr = work.tile([1, 1], F32, tag="fdenr")
            nc.vector.reciprocal(fdenr[:], fsum[:])
            elsc = work.tile([1, 1], F32, tag="elsc")
            nc.scalar.activation(elsc[:], lsc_t[:], AF.Exp)
            scl = work.tile([1, 1], F32, tag="scl")
            nc.vector.tensor_tensor(scl[:], fdenr[:], elsc[:], ALU.mult)
            fw = work.tile([1, 3], F32, tag="fw")
            nc.vector.tensor_scalar(fw[:], fexp[:], scl[:], None, ALU.mult)
            fw_ps = psF.tile([128, 3], F32)
            nc.tensor.matmul(fw_ps[:], onerf_t[:], fw[:], start=True,
                             stop=True)
            nc.vector.tensor_copy(fwc[:], fw_ps[:])

        # ---------------- main loop over the 4 query slices
        obuf = persist.tile([128, 4 * G * K], F32)
        simpool = ctx.enter_context(tc.tile_pool(name="simpool", bufs=2))
        winpool = ctx.enter_context(tc.tile_pool(name="winpool", bufs=2))
        psM = ctx.enter_context(tc.tile_pool(name="psM", bufs=3, space="PSUM"))
        psS = ctx.enter_context(tc.tile_pool(name="psS", bufs=2, space="PSUM"))

        for g in range(G):
            tfg = tf_t[g][:].rearrange("p (j o t q) -> p j o t q", j=DJ, o=2,
                                       t=T)
            tsg = tseg_t[g][:].rearrange("p (c w q) -> p c w q", c=DCH, w=NW)

            # sims: [q, tq(8), ts(8), k(64)] bf16, true cosine values
            simcp = simpool.tile([128, T * T * K], BF16, tag="simcp")
            simv = simcp[:].rearrange("p (t s k) -> p t s k", t=T, s=T)
            for tq in range(T):
                mp = psM.tile([128, T * K], F32, tag="mp")
                for j in range(DJ):
                    nc.tensor.matmul(
                        mp[:], tfg[:, j, :, tq, :], protoT_v[:, j, :, :, :],
                        start=(j == 0), stop=(j == DJ - 1), perf_mode=DR)
                nc.scalar.activation(
                    simcp[:, tq * T * K:(tq + 1) * T * K], mp[:],
                    AF.Copy, scale=rqv_t[g][:, tq:tq + 1])

            # segment sims: [q, wq(3), ws(3), k(64)] bf16, normalized
            wins = winpool.tile([128, NW * NW * K], BF16, tag="wins")
            winv = wins[:].rearrange("p (v w k) -> p v w k", v=NW, w=NW)
            for wq in range(NW):
                sp = psS.tile([128, T * K], F32, tag="sp")
                for dch in range(DCH):
                    nc.tensor.matmul(
                        sp[:, 0:NW * K], tsg[:, dch, wq, :],
                        npsegT_v[:, dch, :, :], start=(dch == 0),
                        stop=(dch == DCH - 1))
                nc.scalar.activation(
                    wins[:, wq * NW * K:(wq + 1) * NW * K],
                    sp[:, 0:NW * K], AF.Copy, scale=rqs_t[g][:, wq:wq + 1])

            # ---- frame-level chamfer: halving max/add trees (bf16, 2x DVE)
            # dir2: max over tq (contiguous halves), then sum over ts
            m1 = work.tile([128, 2048], BF16, tag="m1")
            nc.vector.tensor_tensor(m1[:], simcp[:, 0:2048],
                                    simcp[:, 2048:4096], ALU.max)
            m2 = work.tile([128, 1024], BF16, tag="m2")
            nc.vector.tensor_tensor(m2[:], m1[:, 0:1024], m1[:, 1024:2048],
                                    ALU.max)
            mmax = work.tile([128, 512], BF16, tag="mmax")
            nc.vector.tensor_tensor(mmax[:], m2[:, 0:512], m2[:, 512:1024],
                                    ALU.max)
            s1 = work.tile([128, 256], BF16, tag="s1")
            nc.vector.tensor_tensor(s1[:], mmax[:, 0:256], mmax[:, 256:512],
                                    ALU.add)
            s2 = work.tile([128, 128], BF16, tag="s2")
            nc.vector.tensor_tensor(s2[:], s1[:, 0:128], s1[:, 128:256],
                                    ALU.add)
            msum = work.tile([128, K], BF16, tag="msum")
            nc.vector.tensor_tensor(msum[:], s2[:, 0:K], s2[:, K:128],
                                    ALU.add)
            # dir1: max over ts within each tq (strided), then sum over tq
            a1 = work.tile([128, 2048], BF16, tag="a1")
            a1v = a1[:].rearrange("p (t s k) -> p t s k", t=T, s=4)
            nc.vector.tensor_tensor(a1v, simv[:, :, 0:4, :],
                                    simv[:, :, 4:8, :], ALU.max)
            a2 = work.tile([128, 1024], BF16, tag="a2")
            a2v = a2[:].rearrange("p (t s k) -> p t s k", t=T, s=2)
            nc.vector.tensor_tensor(a2v, a1v[:, :, 0:2, :], a1v[:, :, 2:4, :],
                                    ALU.max)
            amax = work.tile([128, 512], BF16, tag="amax")
            amaxv = amax[:].rearrange("p (t k) -> p t k", t=T)
            nc.vector.tensor_tensor(amaxv, a2v[:, :, 0, :], a2v[:, :, 1, :],
                                    ALU.max)
            b1 = work.tile([128, 256], BF16, tag="b1")
            nc.vector.tensor_tensor(b1[:], amax[:, 0:256], amax[:, 256:512],
                                    ALU.add)
            b2 = work.tile([128, 128], BF16, tag="b2")
            nc.vector.tensor_tensor(b2[:], b1[:, 0:128], b1[:, 128:256],
                                    ALU.add)
            asum = work.tile([128, K], BF16, tag="asum")
            nc.vector.tensor_tensor(asum[:], b2[:, 0:K], b2[:, K:128],
                                    ALU.add)
            # -global = asum + msum - 16
            oglo = obuf[:, (G + g) * K:(G + g + 1) * K]
            nc.vector.scalar_tensor_tensor(
                oglo, in0=asum[:], scalar=-16.0, in1=msum[:],
                op0=ALU.add, op1=ALU.add)

            # ---- segment-level chamfer (tiny trees on wins)
            # q2s: max over ws within wq, sum over wq
            sa = work.tile([128, NW * K], BF16, tag="sa")
            sav = sa[:].rearrange("p (v k) -> p v k", v=NW)
            nc.vector.tensor_tensor(sav, winv[:, :, 0, :], winv[:, :, 1, :],
                                    ALU.max)
            nc.vector.tensor_tensor(sav, sav, winv[:, :, 2, :], ALU.max)
            st = work.tile([128, K], BF16, tag="st")
            nc.vector.tensor_tensor(st[:], sa[:, 0:K], sa[:, K:2 * K],
                                    ALU.add)
            oq2s = obuf[:, (3 * G + g) * K:(3 * G + g + 1) * K]
            nc.vector.scalar_tensor_tensor(
                oq2s, in0=sa[:, 2 * K:3 * K], scalar=-3.0, in1=st[:],
                op0=ALU.add, op1=ALU.add)
            # s2q: max over wq, sum over ws
            sm = work.tile([128, NW * K], BF16, tag="sm")
            nc.vector.tensor_tensor(sm[:], wins[:, 0:NW * K],
                                    wins[:, NW * K:2 * NW * K], ALU.max)
            nc.vector.tensor_tensor(sm[:], sm[:],
                                    wins[:, 2 * NW * K:3 * NW * K], ALU.max)
            st2 = work.tile([128, K], BF16, tag="st2")
            nc.vector.tensor_tensor(st2[:], sm[:, 0:K], sm[:, K:2 * K],
                                    ALU.add)
            os2q = obuf[:, (2 * G + g) * K:(2 * G + g + 1) * K]
            nc.vector.scalar_tensor_tensor(
                os2q, in0=sm[:, 2 * K:3 * K], scalar=-3.0, in1=st2[:],
                op0=ALU.add, op1=ALU.add)

            # ---- fused: f0*oglo + f1*os2q + f2*oq2s
            tmp0 = work.tile([128, K], F32, tag="tmp0")
            nc.vector.tensor_scalar(tmp0[:], oglo, fwc[:, 0:1], None,
                                    ALU.mult)
            tmp1 = work.tile([128, K], F32, tag="tmp1")
            nc.vector.scalar_tensor_tensor(
                tmp1[:], in0=os2q, scalar=fwc[:, 1:2], in1=tmp0[:],
                op0=ALU.mult, op1=ALU.add)
            ofus = obuf[:, g * K:(g + 1) * K]
            nc.vector.scalar_tensor_tensor(
                ofus, in0=oq2s, scalar=fwc[:, 2:3], in1=tmp1[:],
                op0=ALU.mult, op1=ALU.add)

        nc.gpsimd.dma_start(oall[:], obuf[:])

    nc.compile()
    return nc


_NC_CACHE = None


def _get_nc():
    global _NC_CACHE
    if _NC_CACHE is None:
        _NC_CACHE = build_nc()
    return _NC_CACHE


# ------------------------------------------------------------------ host side
def build_in_maps(support_features, target_features, support_labels,
                  logit_scale, fusion_logits):
    support_features = np.asarray(support_features, dtype=np.float32)
    target_features = np.asarray(target_features, dtype=np.float32)
    support_labels = np.asarray(support_labels, dtype=np.int32)
    logit_scale = np.asarray(logit_scale, dtype=np.float32)
    fusion_logits = np.asarray(fusion_logits, dtype=np.float32)

    # ---- support: fp8 cast, s-major [c, s, (dch, t, d128)]
    s8 = support_features.astype(NP_F8)                    # [256, 8, 1024]
    sf_h = np.ascontiguousarray(
        s8.reshape(SCH, 128, T, DCH, 128).transpose(0, 1, 3, 2, 4)
    ).reshape(SCH, 128, T * D)

    # ---- replicated proto norm rows (from the same fp8 values the device
    # accumulates, so they match the on-device prototype sums)
    s8f = s8.astype(np.float32)
    proto_sum = np.zeros((K, T, D), np.float32)
    np.add.at(proto_sum, support_labels % K, s8f)
    rp = FSCALE / np.sqrt((proto_sum * proto_sum).sum(-1))   # [K, T]
    segp = np.stack([proto_sum[:, s:e].sum(1) for s, e in WINDOWS], 1)
    rps = FSCALE / np.sqrt((segp * segp).sum(-1))            # [K, NW]
    smallb_h = np.ascontiguousarray(np.broadcast_to(
        np.concatenate([rp.T.reshape(-1), rps.T.reshape(-1)]
                       ).reshape(1, T * K + NW * K),
        (128, T * K + NW * K))).astype(NP_BF16)

    fus_h = fusion_logits.reshape(1, 3)
    lsc_h = logit_scale.reshape(1, 1)
    onerf_h = np.ones((1, 128), np.float32)
    labcols = support_labels.astype(np.float32).reshape(SCH, 128).T  # [128,2]
    kio = np.broadcast_to(np.arange(K, dtype=np.float32).reshape(1, K),
                          (128, K))

    in_maps = []
    for c in range(NCORES):
        x = target_features[c * QPC:(c + 1) * QPC]           # [512, 8, 1024]
        x8 = x.astype(NP_F8)
        x8f = x8.astype(np.float32)
        # d-major per-slice layout [g, p, j, o, t, q]
        tf_h = np.ascontiguousarray(
            x8.reshape(G, 128, T, DJ, 2, 128).transpose(0, 5, 3, 4, 2, 1)
        ).reshape(G, 128, DJ * 2 * T * 128)
        rqv_h = (1.0 / (FSCALE * np.sqrt((x8f * x8f).sum(-1)))
                 ).reshape(G, 128, T)
        # window-summed segments, re-cast to fp8, d-major [g, p, dch, w, q]
        segf = np.stack([x8f[:, s:e].sum(1) for s, e in WINDOWS], 1)
        seg8 = segf.astype(NP_F8)
        seg8f = seg8.astype(np.float32)
        tseg_h = np.ascontiguousarray(
            seg8.reshape(G, 128, NW, DCH, 128).transpose(0, 4, 3, 2, 1)
        ).reshape(G, 128, DCH * NW * 128)
        rqs_h = (1.0 / (FSCALE * np.sqrt((seg8f * seg8f).sum(-1)))
                 ).reshape(G, 128, NW)
        # packed f32 constants: [lab0, lab1, rqv(32), rqs(12), kiota(64)]
        smallf_h = np.ascontiguousarray(np.concatenate(
            [labcols,
             rqv_h.transpose(1, 0, 2).reshape(128, G * T),
             rqs_h.transpose(1, 0, 2).reshape(128, G * NW),
             kio], axis=1).astype(np.float32))
        in_maps.append({
            "tf": tf_h, "tseg": tseg_h, "sf": sf_h,
            "smallf": smallf_h, "smallb": smallb_h,
            "fus": fus_h, "lsc": lsc_h, "onerf": onerf_h,
        })
    return in_maps


def kernel(support_features, target_features, support_labels, logit_scale,
           fusion_logits):
    in_maps = build_in_maps(support_features, target_features, support_labels,
                            logit_scale, fusion_logits)
    nc = _get_nc()
    res = run_bass_kernel_spmd(nc, in_maps, core_ids=list(range(NCORES)))

    outs = []
    for w in range(4):
        parts = []
        for c in range(NCORES):
            blob = np.asarray(res.results[c]["oall"]).reshape(128, 4, G, K)
            # [p, which, g, k] -> queries q = g*128 + p
            parts.append(blob[:, w].transpose(1, 0, 2).reshape(QPC, K))
        outs.append(np.concatenate(parts, axis=0).astype(np.float32))
    return tuple(outs)


if __name__ == "__main__":
    rng = np.random.default_rng(0)
    ins = {
        "support_features": rng.standard_normal((S, T, D), dtype=np.float32),
        "target_features": rng.standard_normal((Q, T, D), dtype=np.float32),
        "support_labels": (np.arange(S) % K).astype(np.int32),
        "logit_scale": np.float32(0.0),
        "fusion_logits": np.zeros(3, np.float32),
    }
    outs = kernel(**ins)
    for o in outs:
        print(o.shape, o.dtype, float(o.mean()))
